# revision 35
# baseline (speedup 1.0000x reference)
"""ConViT (12-layer, H=12, D=64, B=64) forward pass on 8 TRN2 NeuronCores.

Strategy: data-parallel over batch (8 images per core). Host does layout prep
only (im2col of the non-overlapping patch conv, weight transposes to [ci, co],
bf16 casts, LayerNorm affine folding into consumer weights); all FLOPs run
on-device. Activations are feature-major [C -> 6x128 partitions, tokens free];
matmuls are bf16 with f32 PSUM; residual stream in f32.

Perf notes vs v1:
 - LN affine (s, b) folded into qk/v/fc1/head weights on host; V-path bias
   eliminated entirely (softmax rows sum to 1) and absorbed into proj bias.
 - proj/fc2 bias folded into the PSUM->residual scalar_tensor_tensor epilogue.
 - reciprocal_approx_fast instead of iterative reciprocal.
 - single persistent PSUM ring pool (no per-phase pool barriers).
 - attention: merged exp over both key halves; softmax denominator broadcast
   into rows 64:128 of the same PSUM slot (col-tiled matmul); one-head
   lookahead pipeline.
"""
import os
import sys
import types
import contextlib
import ctypes
from contextlib import ExitStack

import numpy as np
import ml_dtypes

import concourse.bass as bass
import concourse.mybir as mybir
import concourse.tile as tile
from concourse import bacc
from concourse.masks import make_identity

F32 = mybir.dt.float32
BF16 = mybir.dt.bfloat16
AF = mybir.ActivationFunctionType
ALU = mybir.AluOpType
BF = ml_dtypes.bfloat16

H = 12
D = 64
C = 768
NCH = 6             # C / 128
PS = 16             # patch size
GRID = 14
NPATCH = 196        # tokens per image in GPSA phase
NTOK = 197          # tokens per image in MHSA phase (cls + patches)
L_G = 3
L_M = 9
SCALE = D ** -0.5
EPS = 1e-5
B_CORE = 8
NCORES = 8
MLPC = 24           # 3072 / 128

_CACHE = {}
FAST_RECIP = False


def _install_ntff_hook():
    """Best-effort: enable NTFF profiling under axon (used when BASS_TRACE=1)."""
    if "antenv.axon_hooks" in sys.modules:
        return
    so_path = "/opt/axon/libaxon_pjrt.so"
    if not os.path.exists(so_path):
        return
    try:
        lib = ctypes.CDLL(so_path)
        if not hasattr(lib, "axon_start_nrt_profile"):
            return
        lib.axon_start_nrt_profile.argtypes = [ctypes.POINTER(ctypes.c_int64), ctypes.c_size_t]
        lib.axon_start_nrt_profile.restype = ctypes.c_int64
        lib.axon_stop_nrt_profile.argtypes = [ctypes.c_char_p]
        lib.axon_stop_nrt_profile.restype = ctypes.c_int64

        @contextlib.contextmanager
        def _hook(output_dir, device_ids):
            import jax
            jax.devices()
            if device_ids:
                ids = (ctypes.c_int64 * len(device_ids))(*device_ids)
                rc = lib.axon_start_nrt_profile(ids, len(device_ids))
            else:
                rc = lib.axon_start_nrt_profile(None, 0)
            if rc != 0:
                raise RuntimeError(f"axon_start_nrt_profile rc={rc}")
            try:
                yield
            finally:
                n = lib.axon_stop_nrt_profile(str(output_dir).encode())
                if n < 0:
                    raise RuntimeError(f"axon_stop_nrt_profile rc={n}")

        mod = types.ModuleType("antenv.axon_hooks")
        mod._hook = _hook
        mod.get_axon_ntff_profile_hook = lambda: mod._hook
        mod.set_axon_ntff_profile_hook = lambda h: setattr(mod, "_hook", h)
        sys.modules["antenv.axon_hooks"] = mod
        import antenv
        antenv.axon_hooks = mod
    except Exception:
        pass


def _build_program():
    nc = bacc.Bacc("TRN2", target_bir_lowering=False, debug=False)

    def din(name, shape, dt):
        return nc.dram_tensor(name, shape, dt, kind="ExternalInput").ap()

    xim = din("xim", [C, B_CORE * NPATCH], BF16)
    pwT = din("pwT", [C, C], BF16)
    posT = din("posT", [C, NPATCH], F32)           # pos_embed + patch_b folded
    cls = din("cls", [C], F32)
    headT = din("headT", [C, 1024], BF16)          # * norm_s folded
    head_b = din("head_b", [128, 8], F32)          # + headT @ norm_b folded

    gl, ml = [], []
    for i in range(L_G):
        gl.append({
            "qkT": din(f"g{i}_qkT", [C, 2 * C], BF16),      # * n1s folded
            "qkb": din(f"g{i}_qkb", [128, 2 * NCH], F32),   # qk_w @ n1b
            "vT": din(f"g{i}_vT", [C, C], BF16),            # * n1s folded
            "projT": din(f"g{i}_projT", [C, C], BF16),
            "projb": din(f"g{i}_projb", [128, NCH], F32),   # + proj_w @ (v_w @ n1b)
            "fc1T": din(f"g{i}_fc1T", [C, 4 * C], BF16),    # * n2s folded
            "fc1b": din(f"g{i}_fc1b", [128, MLPC], F32),    # + fc1_w @ n2b
            "fc2T": din(f"g{i}_fc2T", [4 * C, C], BF16),
            "fc2b": din(f"g{i}_fc2b", [128, NCH], F32),
            "ft": din(f"g{i}_ft", [NPATCH, H * NPATCH], BF16),  # pos-softmax^T
            "omdiv": din(f"g{i}_omdiv", [128, H * 64], BF16),   # 1/(1-sigmoid(gate))
            "g64": din(f"g{i}_g64", [64, H], F32),              # sigmoid(gate) bcast
        })
    for i in range(L_M):
        ml.append({
            "qkvT": din(f"m{i}_qkvT", [C, 3 * C], BF16),    # * n1s folded
            "qkb": din(f"m{i}_qkb", [128, 2 * NCH], F32),   # qk rows of qkv_w @ n1b
            "projT": din(f"m{i}_projT", [C, C], BF16),
            "projb": din(f"m{i}_projb", [128, NCH], F32),   # + proj_w @ (v_w @ n1b)
            "fc1T": din(f"m{i}_fc1T", [C, 4 * C], BF16),
            "fc1b": din(f"m{i}_fc1b", [128, MLPC], F32),
            "fc2T": din(f"m{i}_fc2T", [4 * C, C], BF16),
            "fc2b": din(f"m{i}_fc2b", [128, NCH], F32),
        })

    OUT = nc.dram_tensor("out", [1024, B_CORE], F32, kind="ExternalOutput").ap()

    MIDTAGS = [f"qt{j}" for j in range(NCH)] + [f"kt{j}" for j in range(NCH)]

    with ExitStack() as ctx:
        tc = ctx.enter_context(tile.TileContext(nc))

        consts = ctx.enter_context(tc.tile_pool(name="consts", bufs=1))
        res_p = ctx.enter_context(tc.tile_pool(name="res", bufs=1))
        act_p = ctx.enter_context(tc.tile_pool(name="act", bufs=2))   # xn / OT / xn2
        qk_p = ctx.enter_context(tc.tile_pool(name="qk", bufs=2))     # Q^T/K^T + mids + xb
        v_p = ctx.enter_context(tc.tile_pool(name="vp", bufs=1))      # token-major V
        w_p = ctx.enter_context(tc.tile_pool(name="wp", bufs=1))      # streamed weights
        wsm_p = ctx.enter_context(tc.tile_pool(name="wsm", bufs=2))   # per-layer params
        row_p = ctx.enter_context(tc.tile_pool(name="rows", bufs=2))  # [1, *] stat rows
        rowa_p = ctx.enter_context(tc.tile_pool(name="rowsa", bufs=2))  # attn recip rows
        mrb_p = ctx.enter_context(tc.tile_pool(name="mrb", bufs=2))   # bcast mean/rstd bf16
        tmp_p = ctx.enter_context(tc.tile_pool(name="tmp", bufs=2))   # scratch tiles
        tmp1_p = ctx.enter_context(tc.tile_pool(name="tmp1", bufs=2))  # big f32 scratch
        e_p = ctx.enter_context(tc.tile_pool(name="ep", bufs=4))      # exp tiles
        ft_p = ctx.enter_context(tc.tile_pool(name="ft", bufs=1))     # GPSA pos F^T

        # single persistent PSUM pool: "ps" ring of 6 one-bank slots + "mmv"
        psum = ctx.enter_context(tc.tile_pool(name="psum", bufs=6, space="PSUM"))

        def pslot():
            # exactly one 2KB PSUM bank -> slots stay bank-aligned
            return psum.tile([128, 512], F32, tag="ps", name="ps")

        def pv(t):
            # [128, 2, NTOK] view of a bank slot
            return t[:, 0:2 * NTOK].rearrange("p (b t) -> p b t", t=NTOK)

        def recip(out, in_):
            if FAST_RECIP:
                nc.vector.reciprocal_approx_fast(out=out, in_=in_)
            else:
                nc.vector.reciprocal(out=out, in_=in_)

        def _scalar_act_raw(out, in_, func, bias):
            # InstActivation without bass's Reciprocal/Rsqrt accuracy guard.
            eng = nc.scalar
            ins = [eng.lower_ap(in_)]
            for arg in (bias, 1.0, 0.0):  # bias, scale, alpha
                if isinstance(arg, bass.AP):
                    ins.append(eng.lower_ap(arg))
                else:
                    ins.append(mybir.ImmediateValue(dtype=F32, value=arg))
            return eng.add_instruction(mybir.InstActivation(
                name=eng.bass.get_next_instruction_name(),
                func=func, ins=ins, outs=[eng.lower_ap(out)]))

        def scalar_recip(out, in_):
            return _scalar_act_raw(out, in_, AF.Reciprocal, 0.0)

        def scalar_rsqrt(out, in_, bias_ap):
            return _scalar_act_raw(out, in_, AF.Rsqrt, bias_ap)

        ones_col = consts.tile([128, 1], BF16)
        nc.vector.memset(ones_col, 1.0)
        ones_row = consts.tile([1, 128], BF16)
        nc.vector.memset(ones_row, 1.0)
        ones64 = consts.tile([128, 64], BF16)
        nc.vector.memset(ones64, 1.0)
        eps_sb = consts.tile([128, 1], F32)
        nc.vector.memset(eps_sb, EPS)

        res = [res_p.tile([128, B_CORE, NTOK], F32, tag=f"res{c}", name=f"res{c}") for c in range(NCH)]

        # persistent token-major V tiles
        vtiles = {}
        for bi in range(2):
            for hi in range(2):
                vsb = v_p.tile([128, H, 64], BF16, tag=f"v{bi}{hi}", name=f"v{bi}{hi}")
                vtiles[(bi, hi)] = vsb

        # cls token into res[:, :, 0]
        for c in range(NCH):
            src = cls[c * 128:(c + 1) * 128]
            ap = bass.AP(tensor=src.tensor, offset=src.offset,
                         ap=[list(src.ap[0]), [0, B_CORE], [0, 1]])
            nc.sync.dma_start(out=res[c][:, :, 0:1], in_=ap)

        def load_wT(dram, ncol, tag):
            t = w_p.tile([128, NCH, ncol], BF16, tag=tag)
            nc.sync.dma_start(out=t, in_=dram.rearrange("(c p) n -> p c n", p=128))
            return t

        def load_sm(dram, ncol, tag, dt=F32):
            t = wsm_p.tile([128, ncol], dt, tag=tag)
            nc.sync.dma_start(out=t, in_=dram)
            return t

        # ---- patch embed -------------------------------------------------
        pw_sb = load_wT(pwT, C, "wbig")
        xim_sb = w_p.tile([128, MLPC, C], BF16, tag="wbig2")  # reuse fc2-size slot
        ximv = xim_sb.rearrange("p a b -> p (a b)")[:, 0:NCH * B_CORE * NPATCH] \
            .rearrange("p (c n) -> p c n", c=NCH)
        ximd = xim.rearrange("(c p) n -> p c n", p=128)
        for nv in range(4):
            sl = slice(2 * nv * NPATCH, (2 * nv + 2) * NPATCH)
            eng = nc.sync if nv % 2 == 0 else nc.gpsimd
            eng.dma_start(out=ximv[:, :, sl], in_=ximd[:, :, sl])

        for nv in range(4):
            b0 = 2 * nv
            for mc in range(NCH):
                ps = pslot()
                psf = ps[:, 0:2 * NPATCH]
                for kc in range(NCH):
                    nc.tensor.matmul(
                        psf, pw_sb[:, kc, mc * 128:(mc + 1) * 128],
                        ximv[:, kc, b0 * NPATCH:(b0 + 2) * NPATCH],
                        start=(kc == 0), stop=(kc == NCH - 1))
                nc.scalar.activation(
                    out=res[mc][:, b0:b0 + 2, 1:NTOK],
                    in_=psf.rearrange("p (b t) -> p b t", b=2),
                    func=AF.Identity, scale=1.0)
        for c in range(NCH):
            src = posT[c * 128:(c + 1) * 128]
            ap = bass.AP(tensor=src.tensor, offset=src.offset,
                         ap=[list(src.ap[0]), [0, B_CORE], list(src.ap[1])])
            nc.gpsimd.dma_start(out=res[c][:, :, 1:NTOK], in_=ap, accum_op=ALU.add)

        # ---- helpers -----------------------------------------------------
        def make_ln(xn, t0, tl):
            """res -> xn bf16 normalized (no affine; folded into consumers).
            Returns (ls, bn): ls(pair) emits stats + row chains for image
            pairs 2p,2p+1; bn(pair) emits broadcast + normalize. Callers
            weave these between dense phases so the serial row chain hides
            behind matmul work."""
            ntl = 2 * tl
            chains = {}

            def stats(nv):
                b0 = 2 * nv
                s_ps = pslot()
                q_ps = pslot()
                s_row = s_ps[0:1, 0:ntl]
                q_row = q_ps[0:1, 0:ntl]
                for c in range(NCH):
                    xb = qk_p.tile([128, 2, tl], BF16, tag=MIDTAGS[c], name=f"xb{c}")
                    xq = tmp_p.tile([128, 2, tl], BF16, tag="xq")
                    sl = res[c][:, b0:b0 + 2, t0:t0 + tl]
                    nc.scalar.activation(out=xb, in_=sl, func=AF.Identity)
                    nc.vector.tensor_tensor(out=xq, in0=xb, in1=xb, op=ALU.mult)
                    nc.tensor.matmul(s_row, ones_col, xb.rearrange("p b t -> p (b t)"),
                                     start=(c == 0), stop=(c == NCH - 1))
                    nc.tensor.matmul(q_row, ones_col, xq.rearrange("p b t -> p (b t)"),
                                     start=(c == 0), stop=(c == NCH - 1))
                return s_row, q_row

            def rowchain(s_row, q_row):
                v1 = row_p.tile([1, ntl], F32, tag="v1")
                nc.vector.tensor_scalar_mul(v1, s_row, 1.0 / C)
                meanb = row_p.tile([1, ntl], BF16, tag="meanb")
                nc.scalar.activation(out=meanb, in_=v1, func=AF.Identity)
                nc.vector.tensor_tensor(out=v1, in0=v1, in1=v1, op=ALU.mult)
                nc.vector.scalar_tensor_tensor(out=v1, in0=q_row, scalar=1.0 / C,
                                               in1=v1, op0=ALU.mult, op1=ALU.subtract)
                nc.scalar.activation(out=v1, in_=v1, func=AF.Sqrt,
                                     bias=eps_sb[0:1, :], scale=1.0)
                nc.vector.reciprocal_approx_fast(out=v1, in_=v1)
                rstdb = row_p.tile([1, ntl], BF16, tag="rstdb")
                nc.scalar.activation(out=rstdb, in_=v1, func=AF.Identity)
                return meanb, rstdb

            def bcast_norm(nv, meanb, stdb):
                b0 = 2 * nv
                mR = pslot()
                rR = pslot()
                mRf = mR[:, 0:ntl]
                rRf = rR[:, 0:ntl]
                nc.tensor.matmul(mRf, ones_row, meanb, start=True, stop=True)
                nc.tensor.matmul(rRf, ones_row, stdb, start=True, stop=True)
                mRb = mrb_p.tile([128, 2, tl], BF16, tag="mRb")
                rRb = mrb_p.tile([128, 2, tl], BF16, tag="rRb")
                nc.scalar.activation(out=mRb.rearrange("p b t -> p (b t)"), in_=mRf,
                                     func=AF.Identity)
                nc.scalar.activation(out=rRb.rearrange("p b t -> p (b t)"), in_=rRf,
                                     func=AF.Identity)
                for c in range(NCH):
                    tsub = tmp_p.tile([128, 2, tl], BF16, tag="lnt")
                    nc.vector.tensor_sub(tsub, res[c][:, b0:b0 + 2, t0:t0 + tl], mRb)
                    nc.vector.tensor_tensor(out=xn[c][:, b0:b0 + 2, t0:t0 + tl],
                                            in0=tsub, in1=rRb, op=ALU.mult)

            def ls(pair):
                st = [stats(2 * pair + i) for i in range(2)]
                chains[pair] = [rowchain(s, q) for s, q in st]

            def bn(nv):
                bcast_norm(nv, *chains[nv // 2][nv % 2])

            return ls, bn

        def make_proj(wT_sb, biast, t0, tl, rhs_of):
            """proj(nv): res += (rhs @ W^T) + bias."""
            def proj(nv):
                b0 = 2 * nv
                for mc in range(NCH):
                    ps = pslot()
                    psf = ps[:, 0:2 * tl]
                    for kc in range(NCH):
                        nc.tensor.matmul(
                            psf, wT_sb[:, kc, mc * 128:(mc + 1) * 128], rhs_of(kc, b0),
                            start=(kc == 0), stop=(kc == NCH - 1))
                    sl = res[mc][:, b0:b0 + 2, t0:t0 + tl]
                    nc.vector.scalar_tensor_tensor(
                        out=sl, in0=psf.rearrange("p (b t) -> p b t", b=2),
                        scalar=biast[:, mc:mc + 1], in1=sl,
                        op0=ALU.add, op1=ALU.add)
            return proj

        def make_mlp(L, xn, t0, tl):
            fc1_sb = load_wT(L["fc1T"], 4 * C, "wbig")
            fc1b_sb = load_sm(L["fc1b"], MLPC, "fc1b")
            fc2_sb = w_p.tile([128, MLPC, C], BF16, tag="wbig2")
            nc.sync.dma_start(out=fc2_sb, in_=L["fc2T"].rearrange("(c p) n -> p c n", p=128))
            fc2b_sb = load_sm(L["fc2b"], NCH, "fc2b")

            def mlp_nv(nv):
                b0 = 2 * nv
                mids = []
                for mc in range(MLPC):
                    ps = pslot()
                    psf = ps[:, 0:2 * tl]
                    for kc in range(NCH):
                        nc.tensor.matmul(
                            psf, fc1_sb[:, kc, mc * 128:(mc + 1) * 128],
                            xn[kc][:, b0:b0 + 2, t0:t0 + tl],
                            start=(kc == 0), stop=(kc == NCH - 1))
                    mt = qk_p.tile([128, 2 * NTOK], BF16, tag=MIDTAGS[mc % 12])
                    nc.scalar.activation(out=mt[:, 0:2 * tl], in_=psf, func=AF.Gelu,
                                         bias=fc1b_sb[:, mc:mc + 1], scale=1.0)
                    mids.append(mt)
                for mc in range(NCH):
                    ps = pslot()
                    psf = ps[:, 0:2 * tl]
                    for kc in range(MLPC):
                        nc.tensor.matmul(
                            psf, fc2_sb[:, kc, mc * 128:(mc + 1) * 128],
                            mids[kc][:, 0:2 * tl],
                            start=(kc == 0), stop=(kc == MLPC - 1))
                    sl = res[mc][:, b0:b0 + 2, t0:t0 + tl]
                    nc.vector.scalar_tensor_tensor(
                        out=sl, in0=psf.rearrange("p (b t) -> p b t", b=2),
                        scalar=fc2b_sb[:, mc:mc + 1], in1=sl,
                        op0=ALU.add, op1=ALU.add)
            return mlp_nv

        def make_qa(L, xn, OT, t0, tl, gpsa, pos_ctx):
            """qa(nv): QKV projections -> V build -> attention -> OT."""
            kl = tl - 128
            nkeys = [(0, 128), (128, kl)]
            w_qk = pos_ctx["w_qk"]
            w_v = pos_ctx["w_v"]
            qkb_sb = pos_ctx["qkb"]

            def qa(nv):
                b0 = 2 * nv
                qt = [qk_p.tile([128, 2, NTOK], BF16, tag=f"qt{c}", name=f"qtt{c}") for c in range(NCH)]
                kt = [qk_p.tile([128, 2, NTOK], BF16, tag=f"kt{c}", name=f"ktt{c}") for c in range(NCH)]
                for mc in range(2 * NCH):
                    ps = pslot()
                    psf = ps[:, 0:2 * tl]
                    for kc in range(NCH):
                        nc.tensor.matmul(
                            psf, w_qk[:, kc, mc * 128:(mc + 1) * 128],
                            xn[kc][:, b0:b0 + 2, t0:t0 + tl],
                            start=(kc == 0), stop=(kc == NCH - 1))
                    dst = qt[mc] if mc < NCH else kt[mc - NCH]
                    nc.scalar.activation(
                        out=dst[:, :, 0:tl],
                        in_=psf.rearrange("p (b t) -> p b t", b=2), func=AF.Identity,
                        bias=qkb_sb[:, mc:mc + 1], scale=1.0)
                vt = vtiles
                for bi in range(2):
                    b = b0 + bi
                    for hi, (h0, hl) in enumerate(nkeys):
                        vsb = vt[(bi, hi)]
                        ps = psum.tile([128, C], F32, tag="mmv", bufs=1, name="psv",
                                       padded_shape=[128, 1024])
                        for kc in range(NCH):
                            for c0, cl in ((0, 512), (512, 256)):
                                nc.tensor.matmul(
                                    ps[:hl, c0:c0 + cl],
                                    xn[kc][:, b, t0 + h0:t0 + h0 + hl],
                                    w_v(kc)[:, c0:c0 + cl],
                                    start=(kc == 0), stop=(kc == NCH - 1))
                        nc.scalar.activation(
                            out=vsb[:hl, :, 0:64],
                            in_=ps[:hl].rearrange("p (h d) -> p h d", h=H),
                            func=AF.Identity)

                def avden(h, es):
                    """AV matmuls + ones/omdiv denominator broadcast."""
                    oe = pv(pslot())
                    for bi in range(2):
                        nc.tensor.matmul(oe[0:64, bi, 0:tl], vt[(bi, 0)][:, h, 0:64],
                                         es[bi][:, 0, 0:tl], start=(bi == 0), stop=False)
                        nc.tensor.matmul(oe[0:64, bi, 0:tl], vt[(bi, 1)][:kl, h, 0:64],
                                         es[bi][:kl, 1, 0:tl], start=False, stop=(bi == 1))
                    db = pv(pslot())
                    dlhs = pos_ctx["omdiv"] if gpsa else None
                    for bi in range(2):
                        nc.tensor.matmul(db[0:64, bi, 0:tl],
                                         dlhs[:, h, :] if gpsa else ones64,
                                         es[bi][:, 0, 0:tl], start=(bi == 0), stop=False)
                        nc.tensor.matmul(db[0:64, bi, 0:tl],
                                         dlhs[0:kl, h, :] if gpsa else ones64[0:kl],
                                         es[bi][:kl, 1, 0:tl], start=False, stop=(bi == 1))
                    return oe, db

                def normalize(h, oedb, fp):
                    oe, db = oedb
                    ch, off = h // 2, (h % 2) * 64
                    r_sb = rowa_p.tile([64, 2, NTOK], F32, tag="db", bufs=2)
                    nc.vector.reciprocal_approx_fast(out=r_sb[:, :, 0:tl],
                                                     in_=db[0:64, :, 0:tl])
                    for bi in range(2):
                        b = b0 + bi
                        if gpsa:
                            tf = tmp1_p.tile([64, NTOK], BF16, tag="tf")
                            nc.vector.tensor_tensor(out=tf[:, 0:tl],
                                                    in0=oe[0:64, bi, 0:tl],
                                                    in1=r_sb[:, bi, 0:tl], op=ALU.mult)
                            nc.vector.scalar_tensor_tensor(
                                out=OT[ch][off:off + 64, b, t0:t0 + tl],
                                in0=fp[0:64, bi, 0:tl], scalar=pos_ctx["g64"][:, h:h + 1],
                                in1=tf[:, 0:tl], op0=ALU.mult, op1=ALU.add)
                        else:
                            nc.vector.tensor_tensor(
                                out=OT[ch][off:off + 64, b, t0:t0 + tl],
                                in0=oe[0:64, bi, 0:tl], in1=r_sb[:, bi, 0:tl],
                                op=ALU.mult)

                if gpsa:
                    def stageA(h):
                        ch, off = h // 2, (h % 2) * 64
                        es = []
                        for bi in range(2):
                            s = pv(pslot())
                            nc.tensor.matmul(s[:, 0, 0:tl],
                                             kt[ch][off:off + 64, bi, 0:128],
                                             qt[ch][off:off + 64, bi, 0:tl],
                                             start=True, stop=True)
                            nc.tensor.matmul(s[:kl, 1, 0:tl],
                                             kt[ch][off:off + 64, bi, 128:tl],
                                             qt[ch][off:off + 64, bi, 0:tl],
                                             start=True, stop=True)
                            e = e_p.tile([128, 2, NTOK], BF16, tag="e", name="e",
                                         bufs=8)
                            # rows kl:128 of the second half are stale garbage;
                            # never read downstream.
                            nc.scalar.activation(out=e[:, :, 0:tl], in_=s[:, :, 0:tl],
                                                 func=AF.Exp, scale=SCALE)
                            es.append(e)
                        return es

                    def stageB(h, es):
                        FT = pos_ctx["FT"]
                        fp = pv(pslot())
                        for bi in range(2):
                            nc.tensor.matmul(fp[0:64, bi, 0:tl], vt[(bi, 0)][:, h, 0:64],
                                             FT[0][:, h, :], start=True, stop=False)
                            nc.tensor.matmul(fp[0:64, bi, 0:tl], vt[(bi, 1)][:kl, h, 0:64],
                                             FT[1][:kl, h, :], start=False, stop=True)
                        oe = avden(h, es)
                        normalize(h, oe, fp)

                    prev = None
                    for h in range(H):
                        es = stageA(h)
                        if prev is not None:
                            stageB(*prev)
                        prev = (h, es)
                    stageB(*prev)
                else:
                    # MHSA: head pairs (2j, 2j+1) live at partition offsets 0/64
                    # of chunk j -> row-tiled score matmuls run concurrently.
                    def stageA(j):
                        ss = {}
                        for idx in range(2):
                            for bi in range(2):
                                ss[(idx, bi)] = pv(pslot())
                        for bi in range(2):
                            for ci, (c0, cl_) in enumerate(((0, 128), (128, kl))):
                                for idx, off in ((0, 0), (1, 64)):
                                    s = ss[(idx, bi)]
                                    nc.tensor.matmul(
                                        s[0:cl_, ci, 0:tl],
                                        kt[j][off:off + 64, bi, c0:c0 + cl_],
                                        qt[j][off:off + 64, bi, 0:tl],
                                        start=True, stop=True)
                        es = {}
                        for idx in range(2):
                            for bi in range(2):
                                e = e_p.tile([128, 2, NTOK], BF16, tag="e", name="e",
                                             bufs=8)
                                nc.scalar.activation(out=e[:, :, 0:tl],
                                                     in_=ss[(idx, bi)][:, :, 0:tl],
                                                     func=AF.Exp, scale=SCALE)
                                es[(idx, bi)] = e
                        return es

                    def stageB(j, es):
                        for idx in range(2):
                            h = 2 * j + idx
                            epair = [es[(idx, 0)], es[(idx, 1)]]
                            oe = avden(h, epair)
                            normalize(h, oe, None)

                    prev = None
                    for j in range(H // 2):
                        es = stageA(j)
                        if prev is not None:
                            stageB(*prev)
                        prev = (j, es)
                    stageB(*prev)

            return qa

        def make_ln1(gpsa):
            t0, tl = (1, NPATCH) if gpsa else (0, NTOK)
            xn = [act_p.tile([128, B_CORE, NTOK], BF16, tag=f"act{c}", name=f"xn{c}")
                  for c in range(NCH)]
            ls1, bn1 = make_ln(xn, t0, tl)
            return xn, ls1, bn1

        def emit_layer(L, gpsa, pre_ln1, next_gpsa):
            """Emit one transformer layer, weaving LN stages between dense
            phases so their serial row chains hide behind PE matmul work.
            pre_ln1: (xn, ls1, bn1) with ls1(0) and bn1(0) already emitted
            by the previous layer's tail; bn1(1..3)/ls1(1) still pending.
            next_gpsa: None at the last layer, else next layer's gpsa flag;
            returns next layer's pre_ln1."""
            t0, tl = (1, NPATCH) if gpsa else (0, NTOK)
            projb_sb = load_sm(L["projb"], NCH, "projb")
            qkb_sb = load_sm(L["qkb"], 2 * NCH, "qkb")

            pos_ctx = {"qkb": qkb_sb}
            if gpsa:
                pos_ctx["w_qk"] = load_wT(L["qkT"], 2 * C, "wbig")
                v_sb = load_wT(L["vT"], C, "wbig2")
                pos_ctx["w_v"] = lambda kc: v_sb[:, kc, :]
                # host-precomputed pos-softmax^T [key m, head, query n]
                FT = [ft_p.tile([128, H, NPATCH], BF16, tag=f"ft{i}", name=f"ft{i}") for i in range(2)]
                pos_ctx["FT"] = FT
                ftd = L["ft"].rearrange("m (h n) -> m h n", h=H)
                nc.sync.dma_start(out=FT[0][:128], in_=ftd[0:128])
                nc.sync.dma_start(out=FT[1][:68], in_=ftd[128:196])
                omdiv = wsm_p.tile([128, H, 64], BF16, tag="omdiv")
                nc.sync.dma_start(out=omdiv, in_=L["omdiv"].rearrange("p (h d) -> p h d", h=H))
                pos_ctx["omdiv"] = omdiv
                g64 = wsm_p.tile([64, H], F32, tag="g64")
                nc.sync.dma_start(out=g64, in_=L["g64"])
                pos_ctx["g64"] = g64
            else:
                qkv_sb = load_wT(L["qkvT"], 3 * C, "wbig")
                pos_ctx["w_qk"] = qkv_sb
                pos_ctx["w_v"] = lambda kc: qkv_sb[:, kc, 2 * C:3 * C]

            if pre_ln1 is None:
                xn, ls1, bn1 = make_ln1(gpsa)
                ls1(0)
                bn1(0)
                bn1(1)
                ls1(1)
                bn1(2)
                bn1(3)
            else:
                xn, ls1, bn1 = pre_ln1

            OT = [act_p.tile([128, B_CORE, NTOK], BF16, tag=f"act{c}", name=f"ot{c}") for c in range(NCH)]
            qa = make_qa(L, xn, OT, t0, tl, gpsa, pos_ctx)

            def mkprj():
                proj_sb = load_wT(L["projT"], C, "wbig2")
                return make_proj(proj_sb, projb_sb, t0, tl,
                                 lambda kc, b0: OT[kc][:, b0:b0 + 2, t0:t0 + tl])

            qa(0)
            if pre_ln1 is not None:
                bn1(2)
                bn1(3)
            if not gpsa:
                # proj weights go to wbig2 (free since last layer's fc2)
                prj = mkprj()
                qa(1)
                qa(2)
                prj(0)
                qa(3)
                prj(1)
            else:
                qa(1)
                qa(2)
                qa(3)
                # wbig2 holds vT until the last v_build; reload with projT now
                prj = mkprj()
                prj(0)
                prj(1)

            xn2 = [act_p.tile([128, B_CORE, NTOK], BF16, tag=f"act{c}", name=f"xn2_{c}") for c in range(NCH)]
            ls2, bn2 = make_ln(xn2, t0, tl)
            ls2(0)
            prj(2)
            bn2(0)
            bn2(1)
            prj(3)
            ls2(1)
            mlp_nv = make_mlp(L, xn2, t0, tl)
            mlp_nv(0)
            bn2(2)
            bn2(3)
            mlp_nv(1)
            if next_gpsa is None:
                mlp_nv(2)
                mlp_nv(3)
                return None
            nxt = make_ln1(next_gpsa)
            nxt[1](0)          # ls1(0) of next layer
            mlp_nv(2)
            nxt[2](0)          # bn1 nv0
            nxt[2](1)          # bn1 nv1
            mlp_nv(3)
            nxt[1](1)          # ls1(1)
            return nxt

        layers = [(L, True) for L in gl] + [(L, False) for L in ml]
        pre = None
        for i, (L, gpsa) in enumerate(layers):
            nxt_gpsa = layers[i + 1][1] if i + 1 < len(layers) else None
            pre = emit_layer(L, gpsa, pre, nxt_gpsa)

        # ---- final LN on cls + head -------------------------------------
        hw_sb = w_p.tile([128, NCH, 1024], BF16, tag="wbig")
        nc.sync.dma_start(out=hw_sb, in_=headT.rearrange("(c p) n -> p c n", p=128))
        hb_sb = load_sm(head_b, 8, "fc1b")

        s_ps = pslot()
        q_ps = pslot()
        s_row = s_ps[0:1, 0:B_CORE]
        q_row = q_ps[0:1, 0:B_CORE]
        xbs = []
        for c in range(NCH):
            xb = tmp_p.tile([128, B_CORE], BF16, tag="fxb", name=f"fxb{c}", bufs=6)
            xq = tmp_p.tile([128, B_CORE], BF16, tag="fxq")
            sl = res[c][:, :, 0]
            nc.scalar.activation(out=xb, in_=sl, func=AF.Identity)
            nc.vector.tensor_tensor(out=xq, in0=xb, in1=xb, op=ALU.mult)
            nc.tensor.matmul(s_row, ones_col, xb, start=(c == 0), stop=(c == NCH - 1))
            nc.tensor.matmul(q_row, ones_col, xq, start=(c == 0), stop=(c == NCH - 1))
            xbs.append(xb)
        v1 = row_p.tile([1, B_CORE], F32, tag="fv1")
        nc.vector.tensor_scalar_mul(v1, s_row, 1.0 / C)
        meanb = row_p.tile([1, B_CORE], BF16, tag="fmeanb")
        nc.scalar.activation(out=meanb, in_=v1, func=AF.Identity)
        nc.vector.tensor_tensor(out=v1, in0=v1, in1=v1, op=ALU.mult)
        nc.vector.scalar_tensor_tensor(out=v1, in0=q_row, scalar=1.0 / C,
                                       in1=v1, op0=ALU.mult, op1=ALU.subtract)
        nc.scalar.activation(out=v1, in_=v1, func=AF.Sqrt, bias=eps_sb[0:1, :], scale=1.0)
        recip(v1, v1)
        rstdb = row_p.tile([1, B_CORE], BF16, tag="frstdb")
        nc.vector.tensor_copy(out=rstdb, in_=v1)
        mR = pslot()
        rR = pslot()
        nc.tensor.matmul(mR[:, 0:B_CORE], ones_row, meanb, start=True, stop=True)
        nc.tensor.matmul(rR[:, 0:B_CORE], ones_row, rstdb, start=True, stop=True)
        mRb = mrb_p.tile([128, B_CORE], BF16, tag="fmRb")
        rRb = mrb_p.tile([128, B_CORE], BF16, tag="frRb")
        nc.scalar.activation(out=mRb, in_=mR[:, 0:B_CORE], func=AF.Identity)
        nc.scalar.activation(out=rRb, in_=rR[:, 0:B_CORE], func=AF.Identity)
        xnf = []
        for c in range(NCH):
            tsub = tmp_p.tile([128, B_CORE], BF16, tag="flt")
            nc.vector.tensor_sub(tsub, xbs[c], mRb)
            xc = tmp_p.tile([128, B_CORE], BF16, tag="fxn", name=f"fxn{c}", bufs=6)
            nc.vector.tensor_tensor(out=xc, in0=tsub, in1=rRb, op=ALU.mult)
            xnf.append(xc)
        for mc in range(8):
            ps = pslot()
            psv = ps[:, 0:B_CORE]
            for kc in range(NCH):
                nc.tensor.matmul(psv, hw_sb[:, kc, mc * 128:(mc + 1) * 128], xnf[kc],
                                 start=(kc == 0), stop=(kc == NCH - 1))
            ot = tmp_p.tile([128, B_CORE], F32, tag="fout")
            nc.scalar.activation(out=ot, in_=psv, func=AF.Identity,
                                 bias=hb_sb[:, mc:mc + 1], scale=1.0)
            nc.sync.dma_start(out=OUT[mc * 128:(mc + 1) * 128, :], in_=ot)

    nc.compile()
    return nc


# ---------------------------------------------------------------------------
# host side
# ---------------------------------------------------------------------------

def _rel_nkm():
    ind = np.arange(GRID)[None, :] - np.arange(GRID)[:, None]
    indx = np.tile(ind, (GRID, GRID)).astype(np.float32)
    indy = np.repeat(np.repeat(ind, GRID, axis=0), GRID, axis=1).astype(np.float32)
    indd = indx ** 2 + indy ** 2
    rel = np.stack([indx, indy, indd], axis=0)           # [3, n, m]
    return np.ascontiguousarray(rel.transpose(1, 0, 2))  # [n, 3, m]


def _pcol(v, parts=128):
    v = np.asarray(v, np.float32).reshape(-1, parts)
    return np.ascontiguousarray(v.T)


def _wT(w):
    return np.ascontiguousarray(np.asarray(w, np.float32).T.astype(BF))


def _prep_weights(i):
    d = {}
    d["pwT"] = _wT(np.asarray(i["patch_w"], np.float32).reshape(C, C))
    # pos_embed with patch_b folded in
    posT = np.asarray(i["pos_embed"], np.float32)[0].T.copy()   # [C, N]
    posT += np.asarray(i["patch_b"], np.float32)[:, None]
    d["posT"] = np.ascontiguousarray(posT)
    d["cls"] = np.asarray(i["cls_token"], np.float32).reshape(C)
    ns = np.asarray(i["norm_s"], np.float32)
    nb = np.asarray(i["norm_b"], np.float32)
    hw = np.asarray(i["head_w"], np.float32)               # [1000, C]
    hT = np.zeros((C, 1024), np.float32)
    hT[:, :1000] = (hw * ns[None, :]).T
    d["headT"] = np.ascontiguousarray(hT.astype(BF))
    hb = np.zeros(1024, np.float32)
    hb[:1000] = np.asarray(i["head_b"], np.float32) + hw @ nb
    d["head_b"] = _pcol(hb)
    for l in range(L_G):
        s1 = np.asarray(i["g_norm1_s"][l], np.float32)
        b1 = np.asarray(i["g_norm1_b"][l], np.float32)
        s2 = np.asarray(i["g_norm2_s"][l], np.float32)
        b2 = np.asarray(i["g_norm2_b"][l], np.float32)
        qk = np.asarray(i["g_qk_w"][l], np.float32)        # [2C, C]
        vw = np.asarray(i["g_v_w"][l], np.float32)         # [C, C]
        pw = np.asarray(i["g_proj_w"][l], np.float32)      # [C, C]
        fc1 = np.asarray(i["g_fc1_w"][l], np.float32)      # [4C, C]
        d[f"g{l}_qkT"] = _wT(qk * s1[None, :])
        d[f"g{l}_qkb"] = _pcol(qk @ b1)
        d[f"g{l}_vT"] = _wT(vw * s1[None, :])
        d[f"g{l}_projT"] = _wT(pw)
        d[f"g{l}_projb"] = _pcol(np.asarray(i["g_proj_b"][l], np.float32) + pw @ (vw @ b1))
        d[f"g{l}_fc1T"] = _wT(fc1 * s2[None, :])
        d[f"g{l}_fc1b"] = _pcol(np.asarray(i["g_fc1_b"][l], np.float32) + fc1 @ b2)
        d[f"g{l}_fc2T"] = _wT(i["g_fc2_w"][l])
        d[f"g{l}_fc2b"] = _pcol(i["g_fc2_b"][l])
        # pos-softmax (weight-only): scores[h,n,m] = sum_k rel[n,k,m]*posw[h,k]
        relnkm = _rel_nkm()
        posw = np.asarray(i["g_pos_w"][l], np.float32)
        sc = np.einsum('nkm,hk->hnm', relnkm, posw)
        sc -= sc.max(axis=-1, keepdims=True)
        e = np.exp(sc)
        pos = e / e.sum(axis=-1, keepdims=True)            # [H, N, M]
        ftT = np.ascontiguousarray(pos.transpose(2, 0, 1).reshape(NPATCH, H * NPATCH)
                                   .astype(BF))            # [M, H*N]
        d[f"g{l}_ft"] = ftT
        sig = 1.0 / (1.0 + np.exp(-np.asarray(i["g_gate"][l], np.float32)))
        d[f"g{l}_omdiv"] = np.ascontiguousarray(
            np.tile(np.repeat(1.0 / (1.0 - sig), 64).reshape(1, H * 64), (128, 1)).astype(BF))
        d[f"g{l}_g64"] = np.ascontiguousarray(np.tile(sig.reshape(1, H), (64, 1)))
    for l in range(L_M):
        s1 = np.asarray(i["m_norm1_s"][l], np.float32)
        b1 = np.asarray(i["m_norm1_b"][l], np.float32)
        s2 = np.asarray(i["m_norm2_s"][l], np.float32)
        b2 = np.asarray(i["m_norm2_b"][l], np.float32)
        qkv = np.asarray(i["m_qkv_w"][l], np.float32)      # [3C, C]
        pw = np.asarray(i["m_proj_w"][l], np.float32)
        fc1 = np.asarray(i["m_fc1_w"][l], np.float32)
        d[f"m{l}_qkvT"] = _wT(qkv * s1[None, :])
        d[f"m{l}_qkb"] = _pcol(qkv[:2 * C] @ b1)
        d[f"m{l}_projT"] = _wT(pw)
        d[f"m{l}_projb"] = _pcol(np.asarray(i["m_proj_b"][l], np.float32)
                                 + pw @ (qkv[2 * C:] @ b1))
        d[f"m{l}_fc1T"] = _wT(fc1 * s2[None, :])
        d[f"m{l}_fc1b"] = _pcol(np.asarray(i["m_fc1_b"][l], np.float32) + fc1 @ b2)
        d[f"m{l}_fc2T"] = _wT(i["m_fc2_w"][l])
        d[f"m{l}_fc2b"] = _pcol(i["m_fc2_b"][l])
    return d


_last_results = None


def build_in_maps(inputs):
    wmap = _prep_weights(inputs)
    x = np.asarray(inputs["x"], np.float32)
    in_maps = []
    for core in range(NCORES):
        xs = x[core * B_CORE:(core + 1) * B_CORE]
        xi = xs.reshape(B_CORE, 3, GRID, PS, GRID, PS).transpose(1, 3, 5, 0, 2, 4)
        xi = np.ascontiguousarray(xi.reshape(C, B_CORE * NPATCH).astype(BF))
        m = dict(wmap)
        m["xim"] = xi
        in_maps.append(m)
    return in_maps


def get_program():
    if "nc" not in _CACHE:
        _CACHE["nc"] = _build_program()
    return _CACHE["nc"]


def kernel(**inputs):
    global _last_results
    _install_ntff_hook()
    from concourse import bass_utils

    nc = get_program()
    in_maps = build_in_maps(inputs)
    res = bass_utils.run_bass_kernel_spmd(nc, in_maps, core_ids=list(range(NCORES)))
    _last_results = res
    outs = [r["out"][:1000, :].T for r in res.results]
    return np.ascontiguousarray(np.concatenate(outs, axis=0).astype(np.float32))



# revision 36
# speedup vs baseline: 1.0023x; 1.0023x over previous
"""ConViT (12-layer, H=12, D=64, B=64) forward pass on 8 TRN2 NeuronCores.

Strategy: data-parallel over batch (8 images per core). Host does layout prep
only (im2col of the non-overlapping patch conv, weight transposes to [ci, co],
bf16 casts, LayerNorm affine folding into consumer weights); all FLOPs run
on-device. Activations are feature-major [C -> 6x128 partitions, tokens free];
matmuls are bf16 with f32 PSUM; residual stream in f32.

Perf notes vs v1:
 - LN affine (s, b) folded into qk/v/fc1/head weights on host; V-path bias
   eliminated entirely (softmax rows sum to 1) and absorbed into proj bias.
 - proj/fc2 bias folded into the PSUM->residual scalar_tensor_tensor epilogue.
 - reciprocal_approx_fast instead of iterative reciprocal.
 - single persistent PSUM ring pool (no per-phase pool barriers).
 - attention: merged exp over both key halves; softmax denominator broadcast
   into rows 64:128 of the same PSUM slot (col-tiled matmul); one-head
   lookahead pipeline.
"""
import os
import sys
import types
import contextlib
import ctypes
from contextlib import ExitStack

import numpy as np
import ml_dtypes

import concourse.bass as bass
import concourse.mybir as mybir
import concourse.tile as tile
from concourse import bacc
from concourse.masks import make_identity

F32 = mybir.dt.float32
BF16 = mybir.dt.bfloat16
AF = mybir.ActivationFunctionType
ALU = mybir.AluOpType
BF = ml_dtypes.bfloat16

H = 12
D = 64
C = 768
NCH = 6             # C / 128
PS = 16             # patch size
GRID = 14
NPATCH = 196        # tokens per image in GPSA phase
NTOK = 197          # tokens per image in MHSA phase (cls + patches)
L_G = 3
L_M = 9
SCALE = D ** -0.5
EPS = 1e-5
B_CORE = 8
NCORES = 8
MLPC = 24           # 3072 / 128

_CACHE = {}
FAST_RECIP = False


def _install_ntff_hook():
    """Best-effort: enable NTFF profiling under axon (used when BASS_TRACE=1)."""
    if "antenv.axon_hooks" in sys.modules:
        return
    so_path = "/opt/axon/libaxon_pjrt.so"
    if not os.path.exists(so_path):
        return
    try:
        lib = ctypes.CDLL(so_path)
        if not hasattr(lib, "axon_start_nrt_profile"):
            return
        lib.axon_start_nrt_profile.argtypes = [ctypes.POINTER(ctypes.c_int64), ctypes.c_size_t]
        lib.axon_start_nrt_profile.restype = ctypes.c_int64
        lib.axon_stop_nrt_profile.argtypes = [ctypes.c_char_p]
        lib.axon_stop_nrt_profile.restype = ctypes.c_int64

        @contextlib.contextmanager
        def _hook(output_dir, device_ids):
            import jax
            jax.devices()
            if device_ids:
                ids = (ctypes.c_int64 * len(device_ids))(*device_ids)
                rc = lib.axon_start_nrt_profile(ids, len(device_ids))
            else:
                rc = lib.axon_start_nrt_profile(None, 0)
            if rc != 0:
                raise RuntimeError(f"axon_start_nrt_profile rc={rc}")
            try:
                yield
            finally:
                n = lib.axon_stop_nrt_profile(str(output_dir).encode())
                if n < 0:
                    raise RuntimeError(f"axon_stop_nrt_profile rc={n}")

        mod = types.ModuleType("antenv.axon_hooks")
        mod._hook = _hook
        mod.get_axon_ntff_profile_hook = lambda: mod._hook
        mod.set_axon_ntff_profile_hook = lambda h: setattr(mod, "_hook", h)
        sys.modules["antenv.axon_hooks"] = mod
        import antenv
        antenv.axon_hooks = mod
    except Exception:
        pass


def _build_program():
    nc = bacc.Bacc("TRN2", target_bir_lowering=False, debug=False)

    def din(name, shape, dt):
        return nc.dram_tensor(name, shape, dt, kind="ExternalInput").ap()

    xim = din("xim", [C, B_CORE * NPATCH], BF16)
    pwT = din("pwT", [C, C], BF16)
    posT = din("posT", [C, NPATCH], F32)           # pos_embed + patch_b folded
    cls = din("cls", [C], F32)
    headT = din("headT", [C, 1024], BF16)          # * norm_s folded
    head_b = din("head_b", [128, 8], F32)          # + headT @ norm_b folded

    gl, ml = [], []
    for i in range(L_G):
        gl.append({
            "qkT": din(f"g{i}_qkT", [C, 2 * C], BF16),      # * n1s folded
            "qkb": din(f"g{i}_qkb", [128, 2 * NCH], F32),   # qk_w @ n1b
            "vT": din(f"g{i}_vT", [C, C], BF16),            # * n1s folded
            "projT": din(f"g{i}_projT", [C, C], BF16),
            "projb": din(f"g{i}_projb", [128, NCH], F32),   # + proj_w @ (v_w @ n1b)
            "fc1T": din(f"g{i}_fc1T", [C, 4 * C], BF16),    # * n2s folded
            "fc1b": din(f"g{i}_fc1b", [128, MLPC], F32),    # + fc1_w @ n2b
            "fc2T": din(f"g{i}_fc2T", [4 * C, C], BF16),
            "fc2b": din(f"g{i}_fc2b", [128, NCH], F32),
            "ft": din(f"g{i}_ft", [NPATCH, H * NPATCH], BF16),  # pos-softmax^T
            "omdiv": din(f"g{i}_omdiv", [128, H * 64], BF16),   # 1/(1-sigmoid(gate))
            "g64": din(f"g{i}_g64", [64, H], F32),              # sigmoid(gate) bcast
        })
    for i in range(L_M):
        ml.append({
            "qkvT": din(f"m{i}_qkvT", [C, 3 * C], BF16),    # * n1s folded
            "qkb": din(f"m{i}_qkb", [128, 2 * NCH], F32),   # qk rows of qkv_w @ n1b
            "projT": din(f"m{i}_projT", [C, C], BF16),
            "projb": din(f"m{i}_projb", [128, NCH], F32),   # + proj_w @ (v_w @ n1b)
            "fc1T": din(f"m{i}_fc1T", [C, 4 * C], BF16),
            "fc1b": din(f"m{i}_fc1b", [128, MLPC], F32),
            "fc2T": din(f"m{i}_fc2T", [4 * C, C], BF16),
            "fc2b": din(f"m{i}_fc2b", [128, NCH], F32),
        })

    OUT = nc.dram_tensor("out", [1024, B_CORE], F32, kind="ExternalOutput").ap()

    MIDTAGS = [f"qt{j}" for j in range(NCH)] + [f"kt{j}" for j in range(NCH)]

    with ExitStack() as ctx:
        tc = ctx.enter_context(tile.TileContext(nc))

        consts = ctx.enter_context(tc.tile_pool(name="consts", bufs=1))
        res_p = ctx.enter_context(tc.tile_pool(name="res", bufs=1))
        act_p = ctx.enter_context(tc.tile_pool(name="act", bufs=2))   # xn / OT / xn2
        qk_p = ctx.enter_context(tc.tile_pool(name="qk", bufs=2))     # Q^T/K^T + mids + xb
        v_p = ctx.enter_context(tc.tile_pool(name="vp", bufs=1))      # token-major V
        w_p = ctx.enter_context(tc.tile_pool(name="wp", bufs=1))      # streamed weights
        wsm_p = ctx.enter_context(tc.tile_pool(name="wsm", bufs=2))   # per-layer params
        row_p = ctx.enter_context(tc.tile_pool(name="rows", bufs=2))  # [1, *] stat rows
        rowa_p = ctx.enter_context(tc.tile_pool(name="rowsa", bufs=2))  # attn recip rows
        mrb_p = ctx.enter_context(tc.tile_pool(name="mrb", bufs=2))   # bcast mean/rstd bf16
        tmp_p = ctx.enter_context(tc.tile_pool(name="tmp", bufs=2))   # scratch tiles
        tmp1_p = ctx.enter_context(tc.tile_pool(name="tmp1", bufs=2))  # big f32 scratch
        e_p = ctx.enter_context(tc.tile_pool(name="ep", bufs=4))      # exp tiles
        ft_p = ctx.enter_context(tc.tile_pool(name="ft", bufs=1))     # GPSA pos F^T

        # single persistent PSUM pool: "ps" ring of 6 one-bank slots + "mmv"
        psum = ctx.enter_context(tc.tile_pool(name="psum", bufs=6, space="PSUM"))

        def pslot():
            # exactly one 2KB PSUM bank -> slots stay bank-aligned
            return psum.tile([128, 512], F32, tag="ps", name="ps")

        def pv(t):
            # [128, 2, NTOK] view of a bank slot
            return t[:, 0:2 * NTOK].rearrange("p (b t) -> p b t", t=NTOK)

        def recip(out, in_):
            if FAST_RECIP:
                nc.vector.reciprocal_approx_fast(out=out, in_=in_)
            else:
                nc.vector.reciprocal(out=out, in_=in_)

        def _scalar_act_raw(out, in_, func, bias):
            # InstActivation without bass's Reciprocal/Rsqrt accuracy guard.
            eng = nc.scalar
            ins = [eng.lower_ap(in_)]
            for arg in (bias, 1.0, 0.0):  # bias, scale, alpha
                if isinstance(arg, bass.AP):
                    ins.append(eng.lower_ap(arg))
                else:
                    ins.append(mybir.ImmediateValue(dtype=F32, value=arg))
            return eng.add_instruction(mybir.InstActivation(
                name=eng.bass.get_next_instruction_name(),
                func=func, ins=ins, outs=[eng.lower_ap(out)]))

        def scalar_recip(out, in_):
            return _scalar_act_raw(out, in_, AF.Reciprocal, 0.0)

        def scalar_rsqrt(out, in_, bias_ap):
            return _scalar_act_raw(out, in_, AF.Rsqrt, bias_ap)

        ones_col = consts.tile([128, 1], BF16)
        nc.vector.memset(ones_col, 1.0)
        ones_row = consts.tile([1, 128], BF16)
        nc.vector.memset(ones_row, 1.0)
        ones64 = consts.tile([128, 64], BF16)
        nc.vector.memset(ones64, 1.0)
        eps_sb = consts.tile([128, 1], F32)
        nc.vector.memset(eps_sb, EPS)

        res = [res_p.tile([128, B_CORE, NTOK], F32, tag=f"res{c}", name=f"res{c}") for c in range(NCH)]

        # persistent token-major V tiles
        vtiles = {}
        for bi in range(2):
            for hi in range(2):
                vsb = v_p.tile([128, H, 64], BF16, tag=f"v{bi}{hi}", name=f"v{bi}{hi}")
                vtiles[(bi, hi)] = vsb

        # cls token into res[:, :, 0]
        for c in range(NCH):
            src = cls[c * 128:(c + 1) * 128]
            ap = bass.AP(tensor=src.tensor, offset=src.offset,
                         ap=[list(src.ap[0]), [0, B_CORE], [0, 1]])
            nc.sync.dma_start(out=res[c][:, :, 0:1], in_=ap)

        def load_wT(dram, ncol, tag):
            t = w_p.tile([128, NCH, ncol], BF16, tag=tag)
            nc.sync.dma_start(out=t, in_=dram.rearrange("(c p) n -> p c n", p=128))
            return t

        def load_sm(dram, ncol, tag, dt=F32):
            t = wsm_p.tile([128, ncol], dt, tag=tag)
            nc.sync.dma_start(out=t, in_=dram)
            return t

        # ---- patch embed -------------------------------------------------
        pw_sb = load_wT(pwT, C, "wbig")
        xim_sb = w_p.tile([128, MLPC, C], BF16, tag="wbig2")  # reuse fc2-size slot
        ximv = xim_sb.rearrange("p a b -> p (a b)")[:, 0:NCH * B_CORE * NPATCH] \
            .rearrange("p (c n) -> p c n", c=NCH)
        ximd = xim.rearrange("(c p) n -> p c n", p=128)
        for nv in range(4):
            sl = slice(2 * nv * NPATCH, (2 * nv + 2) * NPATCH)
            nc.sync.dma_start(out=ximv[:, :, sl], in_=ximd[:, :, sl])

        for nv in range(4):
            b0 = 2 * nv
            for mc in range(NCH):
                ps = pslot()
                psf = ps[:, 0:2 * NPATCH]
                for kc in range(NCH):
                    nc.tensor.matmul(
                        psf, pw_sb[:, kc, mc * 128:(mc + 1) * 128],
                        ximv[:, kc, b0 * NPATCH:(b0 + 2) * NPATCH],
                        start=(kc == 0), stop=(kc == NCH - 1))
                nc.scalar.activation(
                    out=res[mc][:, b0:b0 + 2, 1:NTOK],
                    in_=psf.rearrange("p (b t) -> p b t", b=2),
                    func=AF.Identity, scale=1.0)
        for c in range(NCH):
            src = posT[c * 128:(c + 1) * 128]
            ap = bass.AP(tensor=src.tensor, offset=src.offset,
                         ap=[list(src.ap[0]), [0, B_CORE], list(src.ap[1])])
            nc.gpsimd.dma_start(out=res[c][:, :, 1:NTOK], in_=ap, accum_op=ALU.add)

        # ---- helpers -----------------------------------------------------
        def make_ln(xn, t0, tl):
            """res -> xn bf16 normalized (no affine; folded into consumers).
            Returns (ls, bn): ls(pair) emits stats + row chains for image
            pairs 2p,2p+1; bn(pair) emits broadcast + normalize. Callers
            weave these between dense phases so the serial row chain hides
            behind matmul work."""
            ntl = 2 * tl
            chains = {}

            def stats(nv):
                b0 = 2 * nv
                s_ps = pslot()
                q_ps = pslot()
                s_row = s_ps[0:1, 0:ntl]
                q_row = q_ps[0:1, 0:ntl]
                for c in range(NCH):
                    xb = qk_p.tile([128, 2, tl], BF16, tag=MIDTAGS[c], name=f"xb{c}")
                    xq = tmp_p.tile([128, 2, tl], BF16, tag="xq")
                    sl = res[c][:, b0:b0 + 2, t0:t0 + tl]
                    nc.scalar.activation(out=xb, in_=sl, func=AF.Identity)
                    nc.vector.tensor_tensor(out=xq, in0=xb, in1=xb, op=ALU.mult)
                    nc.tensor.matmul(s_row, ones_col, xb.rearrange("p b t -> p (b t)"),
                                     start=(c == 0), stop=(c == NCH - 1))
                    nc.tensor.matmul(q_row, ones_col, xq.rearrange("p b t -> p (b t)"),
                                     start=(c == 0), stop=(c == NCH - 1))
                return s_row, q_row

            def rowchain(s_row, q_row):
                v1 = row_p.tile([1, ntl], F32, tag="v1")
                nc.vector.tensor_scalar_mul(v1, s_row, 1.0 / C)
                meanb = row_p.tile([1, ntl], BF16, tag="meanb")
                nc.scalar.activation(out=meanb, in_=v1, func=AF.Identity)
                nc.vector.tensor_tensor(out=v1, in0=v1, in1=v1, op=ALU.mult)
                nc.vector.scalar_tensor_tensor(out=v1, in0=q_row, scalar=1.0 / C,
                                               in1=v1, op0=ALU.mult, op1=ALU.subtract)
                nc.scalar.activation(out=v1, in_=v1, func=AF.Sqrt,
                                     bias=eps_sb[0:1, :], scale=1.0)
                nc.vector.reciprocal_approx_fast(out=v1, in_=v1)
                rstdb = row_p.tile([1, ntl], BF16, tag="rstdb")
                nc.scalar.activation(out=rstdb, in_=v1, func=AF.Identity)
                return meanb, rstdb

            def bcast_norm(nv, meanb, stdb):
                b0 = 2 * nv
                mR = pslot()
                rR = pslot()
                mRf = mR[:, 0:ntl]
                rRf = rR[:, 0:ntl]
                nc.tensor.matmul(mRf, ones_row, meanb, start=True, stop=True)
                nc.tensor.matmul(rRf, ones_row, stdb, start=True, stop=True)
                mRb = mrb_p.tile([128, 2, tl], BF16, tag="mRb")
                rRb = mrb_p.tile([128, 2, tl], BF16, tag="rRb")
                nc.scalar.activation(out=mRb.rearrange("p b t -> p (b t)"), in_=mRf,
                                     func=AF.Identity)
                nc.scalar.activation(out=rRb.rearrange("p b t -> p (b t)"), in_=rRf,
                                     func=AF.Identity)
                for c in range(NCH):
                    tsub = tmp_p.tile([128, 2, tl], BF16, tag="lnt")
                    nc.vector.tensor_sub(tsub, res[c][:, b0:b0 + 2, t0:t0 + tl], mRb)
                    nc.vector.tensor_tensor(out=xn[c][:, b0:b0 + 2, t0:t0 + tl],
                                            in0=tsub, in1=rRb, op=ALU.mult)

            def ls(pair):
                st = [stats(2 * pair + i) for i in range(2)]
                chains[pair] = [rowchain(s, q) for s, q in st]

            def bn(nv):
                bcast_norm(nv, *chains[nv // 2][nv % 2])

            return ls, bn

        def make_proj(wT_sb, biast, t0, tl, rhs_of):
            """proj(nv): res += (rhs @ W^T) + bias."""
            def proj(nv):
                b0 = 2 * nv
                for mc in range(NCH):
                    ps = pslot()
                    psf = ps[:, 0:2 * tl]
                    for kc in range(NCH):
                        nc.tensor.matmul(
                            psf, wT_sb[:, kc, mc * 128:(mc + 1) * 128], rhs_of(kc, b0),
                            start=(kc == 0), stop=(kc == NCH - 1))
                    sl = res[mc][:, b0:b0 + 2, t0:t0 + tl]
                    nc.vector.scalar_tensor_tensor(
                        out=sl, in0=psf.rearrange("p (b t) -> p b t", b=2),
                        scalar=biast[:, mc:mc + 1], in1=sl,
                        op0=ALU.add, op1=ALU.add)
            return proj

        def make_mlp(L, xn, t0, tl):
            fc1_sb = load_wT(L["fc1T"], 4 * C, "wbig")
            fc1b_sb = load_sm(L["fc1b"], MLPC, "fc1b")
            fc2_sb = w_p.tile([128, MLPC, C], BF16, tag="wbig2")
            nc.sync.dma_start(out=fc2_sb, in_=L["fc2T"].rearrange("(c p) n -> p c n", p=128))
            fc2b_sb = load_sm(L["fc2b"], NCH, "fc2b")

            def mlp_nv(nv):
                b0 = 2 * nv
                mids = []
                for mc in range(MLPC):
                    ps = pslot()
                    psf = ps[:, 0:2 * tl]
                    for kc in range(NCH):
                        nc.tensor.matmul(
                            psf, fc1_sb[:, kc, mc * 128:(mc + 1) * 128],
                            xn[kc][:, b0:b0 + 2, t0:t0 + tl],
                            start=(kc == 0), stop=(kc == NCH - 1))
                    mt = qk_p.tile([128, 2 * NTOK], BF16, tag=MIDTAGS[mc % 12])
                    nc.scalar.activation(out=mt[:, 0:2 * tl], in_=psf, func=AF.Gelu,
                                         bias=fc1b_sb[:, mc:mc + 1], scale=1.0)
                    mids.append(mt)
                for mc in range(NCH):
                    ps = pslot()
                    psf = ps[:, 0:2 * tl]
                    for kc in range(MLPC):
                        nc.tensor.matmul(
                            psf, fc2_sb[:, kc, mc * 128:(mc + 1) * 128],
                            mids[kc][:, 0:2 * tl],
                            start=(kc == 0), stop=(kc == MLPC - 1))
                    sl = res[mc][:, b0:b0 + 2, t0:t0 + tl]
                    nc.vector.scalar_tensor_tensor(
                        out=sl, in0=psf.rearrange("p (b t) -> p b t", b=2),
                        scalar=fc2b_sb[:, mc:mc + 1], in1=sl,
                        op0=ALU.add, op1=ALU.add)
            return mlp_nv

        def make_qa(L, xn, OT, t0, tl, gpsa, pos_ctx):
            """qa(nv): QKV projections -> V build -> attention -> OT."""
            kl = tl - 128
            nkeys = [(0, 128), (128, kl)]
            w_qk = pos_ctx["w_qk"]
            w_v = pos_ctx["w_v"]
            qkb_sb = pos_ctx["qkb"]

            def qa(nv):
                b0 = 2 * nv
                qt = [qk_p.tile([128, 2, NTOK], BF16, tag=f"qt{c}", name=f"qtt{c}") for c in range(NCH)]
                kt = [qk_p.tile([128, 2, NTOK], BF16, tag=f"kt{c}", name=f"ktt{c}") for c in range(NCH)]
                for mc in range(2 * NCH):
                    ps = pslot()
                    psf = ps[:, 0:2 * tl]
                    for kc in range(NCH):
                        nc.tensor.matmul(
                            psf, w_qk[:, kc, mc * 128:(mc + 1) * 128],
                            xn[kc][:, b0:b0 + 2, t0:t0 + tl],
                            start=(kc == 0), stop=(kc == NCH - 1))
                    dst = qt[mc] if mc < NCH else kt[mc - NCH]
                    nc.scalar.activation(
                        out=dst[:, :, 0:tl],
                        in_=psf.rearrange("p (b t) -> p b t", b=2), func=AF.Identity,
                        bias=qkb_sb[:, mc:mc + 1], scale=1.0)
                vt = vtiles
                for bi in range(2):
                    b = b0 + bi
                    for hi, (h0, hl) in enumerate(nkeys):
                        vsb = vt[(bi, hi)]
                        ps = psum.tile([128, C], F32, tag="mmv", bufs=1, name="psv",
                                       padded_shape=[128, 1024])
                        for kc in range(NCH):
                            for c0, cl in ((0, 512), (512, 256)):
                                nc.tensor.matmul(
                                    ps[:hl, c0:c0 + cl],
                                    xn[kc][:, b, t0 + h0:t0 + h0 + hl],
                                    w_v(kc)[:, c0:c0 + cl],
                                    start=(kc == 0), stop=(kc == NCH - 1))
                        nc.scalar.activation(
                            out=vsb[:hl, :, 0:64],
                            in_=ps[:hl].rearrange("p (h d) -> p h d", h=H),
                            func=AF.Identity)

                def avden(h, es):
                    """AV matmuls + ones/omdiv denominator broadcast."""
                    oe = pv(pslot())
                    for bi in range(2):
                        nc.tensor.matmul(oe[0:64, bi, 0:tl], vt[(bi, 0)][:, h, 0:64],
                                         es[bi][:, 0, 0:tl], start=(bi == 0), stop=False)
                        nc.tensor.matmul(oe[0:64, bi, 0:tl], vt[(bi, 1)][:kl, h, 0:64],
                                         es[bi][:kl, 1, 0:tl], start=False, stop=(bi == 1))
                    db = pv(pslot())
                    dlhs = pos_ctx["omdiv"] if gpsa else None
                    for bi in range(2):
                        nc.tensor.matmul(db[0:64, bi, 0:tl],
                                         dlhs[:, h, :] if gpsa else ones64,
                                         es[bi][:, 0, 0:tl], start=(bi == 0), stop=False)
                        nc.tensor.matmul(db[0:64, bi, 0:tl],
                                         dlhs[0:kl, h, :] if gpsa else ones64[0:kl],
                                         es[bi][:kl, 1, 0:tl], start=False, stop=(bi == 1))
                    return oe, db

                def normalize(h, oedb, fp):
                    oe, db = oedb
                    ch, off = h // 2, (h % 2) * 64
                    r_sb = rowa_p.tile([64, 2, NTOK], F32, tag="db", bufs=2)
                    nc.vector.reciprocal_approx_fast(out=r_sb[:, :, 0:tl],
                                                     in_=db[0:64, :, 0:tl])
                    for bi in range(2):
                        b = b0 + bi
                        if gpsa:
                            tf = tmp1_p.tile([64, NTOK], BF16, tag="tf")
                            nc.vector.tensor_tensor(out=tf[:, 0:tl],
                                                    in0=oe[0:64, bi, 0:tl],
                                                    in1=r_sb[:, bi, 0:tl], op=ALU.mult)
                            nc.vector.scalar_tensor_tensor(
                                out=OT[ch][off:off + 64, b, t0:t0 + tl],
                                in0=fp[0:64, bi, 0:tl], scalar=pos_ctx["g64"][:, h:h + 1],
                                in1=tf[:, 0:tl], op0=ALU.mult, op1=ALU.add)
                        else:
                            nc.vector.tensor_tensor(
                                out=OT[ch][off:off + 64, b, t0:t0 + tl],
                                in0=oe[0:64, bi, 0:tl], in1=r_sb[:, bi, 0:tl],
                                op=ALU.mult)

                if gpsa:
                    def stageA(h):
                        ch, off = h // 2, (h % 2) * 64
                        es = []
                        for bi in range(2):
                            s = pv(pslot())
                            nc.tensor.matmul(s[:, 0, 0:tl],
                                             kt[ch][off:off + 64, bi, 0:128],
                                             qt[ch][off:off + 64, bi, 0:tl],
                                             start=True, stop=True)
                            nc.tensor.matmul(s[:kl, 1, 0:tl],
                                             kt[ch][off:off + 64, bi, 128:tl],
                                             qt[ch][off:off + 64, bi, 0:tl],
                                             start=True, stop=True)
                            e = e_p.tile([128, 2, NTOK], BF16, tag="e", name="e",
                                         bufs=8)
                            # rows kl:128 of the second half are stale garbage;
                            # never read downstream.
                            nc.scalar.activation(out=e[:, :, 0:tl], in_=s[:, :, 0:tl],
                                                 func=AF.Exp, scale=SCALE)
                            es.append(e)
                        return es

                    def stageB(h, es):
                        FT = pos_ctx["FT"]
                        fp = pv(pslot())
                        for bi in range(2):
                            nc.tensor.matmul(fp[0:64, bi, 0:tl], vt[(bi, 0)][:, h, 0:64],
                                             FT[0][:, h, :], start=True, stop=False)
                            nc.tensor.matmul(fp[0:64, bi, 0:tl], vt[(bi, 1)][:kl, h, 0:64],
                                             FT[1][:kl, h, :], start=False, stop=True)
                        oe = avden(h, es)
                        normalize(h, oe, fp)

                    prev = None
                    for h in range(H):
                        es = stageA(h)
                        if prev is not None:
                            stageB(*prev)
                        prev = (h, es)
                    stageB(*prev)
                else:
                    # MHSA: head pairs (2j, 2j+1) live at partition offsets 0/64
                    # of chunk j -> row-tiled score matmuls run concurrently.
                    def stageA(j):
                        ss = {}
                        for idx in range(2):
                            for bi in range(2):
                                ss[(idx, bi)] = pv(pslot())
                        for bi in range(2):
                            for ci, (c0, cl_) in enumerate(((0, 128), (128, kl))):
                                for idx, off in ((0, 0), (1, 64)):
                                    s = ss[(idx, bi)]
                                    nc.tensor.matmul(
                                        s[0:cl_, ci, 0:tl],
                                        kt[j][off:off + 64, bi, c0:c0 + cl_],
                                        qt[j][off:off + 64, bi, 0:tl],
                                        start=True, stop=True)
                        es = {}
                        for idx in range(2):
                            for bi in range(2):
                                e = e_p.tile([128, 2, NTOK], BF16, tag="e", name="e",
                                             bufs=8)
                                nc.scalar.activation(out=e[:, :, 0:tl],
                                                     in_=ss[(idx, bi)][:, :, 0:tl],
                                                     func=AF.Exp, scale=SCALE)
                                es[(idx, bi)] = e
                        return es

                    def stageB(j, es):
                        for idx in range(2):
                            h = 2 * j + idx
                            epair = [es[(idx, 0)], es[(idx, 1)]]
                            oe = avden(h, epair)
                            normalize(h, oe, None)

                    prev = None
                    for j in range(H // 2):
                        es = stageA(j)
                        if prev is not None:
                            stageB(*prev)
                        prev = (j, es)
                    stageB(*prev)

            return qa

        def make_ln1(gpsa):
            t0, tl = (1, NPATCH) if gpsa else (0, NTOK)
            xn = [act_p.tile([128, B_CORE, NTOK], BF16, tag=f"act{c}", name=f"xn{c}")
                  for c in range(NCH)]
            ls1, bn1 = make_ln(xn, t0, tl)
            return xn, ls1, bn1

        def emit_layer(L, gpsa, pre_ln1, next_gpsa):
            """Emit one transformer layer, weaving LN stages between dense
            phases so their serial row chains hide behind PE matmul work.
            pre_ln1: (xn, ls1, bn1) with ls1(0) and bn1(0) already emitted
            by the previous layer's tail; bn1(1..3)/ls1(1) still pending.
            next_gpsa: None at the last layer, else next layer's gpsa flag;
            returns next layer's pre_ln1."""
            t0, tl = (1, NPATCH) if gpsa else (0, NTOK)
            projb_sb = load_sm(L["projb"], NCH, "projb")
            qkb_sb = load_sm(L["qkb"], 2 * NCH, "qkb")

            pos_ctx = {"qkb": qkb_sb}
            if gpsa:
                pos_ctx["w_qk"] = load_wT(L["qkT"], 2 * C, "wbig")
                v_sb = load_wT(L["vT"], C, "wbig2")
                pos_ctx["w_v"] = lambda kc: v_sb[:, kc, :]
                # host-precomputed pos-softmax^T [key m, head, query n]
                FT = [ft_p.tile([128, H, NPATCH], BF16, tag=f"ft{i}", name=f"ft{i}") for i in range(2)]
                pos_ctx["FT"] = FT
                ftd = L["ft"].rearrange("m (h n) -> m h n", h=H)
                nc.sync.dma_start(out=FT[0][:128], in_=ftd[0:128])
                nc.sync.dma_start(out=FT[1][:68], in_=ftd[128:196])
                omdiv = wsm_p.tile([128, H, 64], BF16, tag="omdiv")
                nc.sync.dma_start(out=omdiv, in_=L["omdiv"].rearrange("p (h d) -> p h d", h=H))
                pos_ctx["omdiv"] = omdiv
                g64 = wsm_p.tile([64, H], F32, tag="g64")
                nc.sync.dma_start(out=g64, in_=L["g64"])
                pos_ctx["g64"] = g64
            else:
                qkv_sb = load_wT(L["qkvT"], 3 * C, "wbig")
                pos_ctx["w_qk"] = qkv_sb
                pos_ctx["w_v"] = lambda kc: qkv_sb[:, kc, 2 * C:3 * C]

            if pre_ln1 is None:
                xn, ls1, bn1 = make_ln1(gpsa)
                ls1(0)
                bn1(0)
                bn1(1)
                ls1(1)
                bn1(2)
                bn1(3)
            else:
                xn, ls1, bn1 = pre_ln1

            OT = [act_p.tile([128, B_CORE, NTOK], BF16, tag=f"act{c}", name=f"ot{c}") for c in range(NCH)]
            qa = make_qa(L, xn, OT, t0, tl, gpsa, pos_ctx)

            def mkprj():
                proj_sb = load_wT(L["projT"], C, "wbig2")
                return make_proj(proj_sb, projb_sb, t0, tl,
                                 lambda kc, b0: OT[kc][:, b0:b0 + 2, t0:t0 + tl])

            qa(0)
            if pre_ln1 is not None:
                bn1(2)
                bn1(3)
            if not gpsa:
                # proj weights go to wbig2 (free since last layer's fc2)
                prj = mkprj()
                qa(1)
                qa(2)
                prj(0)
                qa(3)
                prj(1)
            else:
                qa(1)
                qa(2)
                qa(3)
                # wbig2 holds vT until the last v_build; reload with projT now
                prj = mkprj()
                prj(0)
                prj(1)

            xn2 = [act_p.tile([128, B_CORE, NTOK], BF16, tag=f"act{c}", name=f"xn2_{c}") for c in range(NCH)]
            ls2, bn2 = make_ln(xn2, t0, tl)
            ls2(0)
            prj(2)
            bn2(0)
            bn2(1)
            prj(3)
            ls2(1)
            mlp_nv = make_mlp(L, xn2, t0, tl)
            mlp_nv(0)
            bn2(2)
            bn2(3)
            mlp_nv(1)
            if next_gpsa is None:
                mlp_nv(2)
                mlp_nv(3)
                return None
            nxt = make_ln1(next_gpsa)
            nxt[1](0)          # ls1(0) of next layer
            mlp_nv(2)
            nxt[2](0)          # bn1 nv0
            nxt[2](1)          # bn1 nv1
            mlp_nv(3)
            nxt[1](1)          # ls1(1)
            return nxt

        layers = [(L, True) for L in gl] + [(L, False) for L in ml]
        pre = None
        for i, (L, gpsa) in enumerate(layers):
            nxt_gpsa = layers[i + 1][1] if i + 1 < len(layers) else None
            pre = emit_layer(L, gpsa, pre, nxt_gpsa)

        # ---- final LN on cls + head -------------------------------------
        hw_sb = w_p.tile([128, NCH, 1024], BF16, tag="wbig")
        nc.sync.dma_start(out=hw_sb, in_=headT.rearrange("(c p) n -> p c n", p=128))
        hb_sb = load_sm(head_b, 8, "fc1b")

        s_ps = pslot()
        q_ps = pslot()
        s_row = s_ps[0:1, 0:B_CORE]
        q_row = q_ps[0:1, 0:B_CORE]
        xbs = []
        for c in range(NCH):
            xb = tmp_p.tile([128, B_CORE], BF16, tag="fxb", name=f"fxb{c}", bufs=6)
            xq = tmp_p.tile([128, B_CORE], BF16, tag="fxq")
            sl = res[c][:, :, 0]
            nc.scalar.activation(out=xb, in_=sl, func=AF.Identity)
            nc.vector.tensor_tensor(out=xq, in0=xb, in1=xb, op=ALU.mult)
            nc.tensor.matmul(s_row, ones_col, xb, start=(c == 0), stop=(c == NCH - 1))
            nc.tensor.matmul(q_row, ones_col, xq, start=(c == 0), stop=(c == NCH - 1))
            xbs.append(xb)
        v1 = row_p.tile([1, B_CORE], F32, tag="fv1")
        nc.vector.tensor_scalar_mul(v1, s_row, 1.0 / C)
        meanb = row_p.tile([1, B_CORE], BF16, tag="fmeanb")
        nc.scalar.activation(out=meanb, in_=v1, func=AF.Identity)
        nc.vector.tensor_tensor(out=v1, in0=v1, in1=v1, op=ALU.mult)
        nc.vector.scalar_tensor_tensor(out=v1, in0=q_row, scalar=1.0 / C,
                                       in1=v1, op0=ALU.mult, op1=ALU.subtract)
        nc.scalar.activation(out=v1, in_=v1, func=AF.Sqrt, bias=eps_sb[0:1, :], scale=1.0)
        recip(v1, v1)
        rstdb = row_p.tile([1, B_CORE], BF16, tag="frstdb")
        nc.vector.tensor_copy(out=rstdb, in_=v1)
        mR = pslot()
        rR = pslot()
        nc.tensor.matmul(mR[:, 0:B_CORE], ones_row, meanb, start=True, stop=True)
        nc.tensor.matmul(rR[:, 0:B_CORE], ones_row, rstdb, start=True, stop=True)
        mRb = mrb_p.tile([128, B_CORE], BF16, tag="fmRb")
        rRb = mrb_p.tile([128, B_CORE], BF16, tag="frRb")
        nc.scalar.activation(out=mRb, in_=mR[:, 0:B_CORE], func=AF.Identity)
        nc.scalar.activation(out=rRb, in_=rR[:, 0:B_CORE], func=AF.Identity)
        xnf = []
        for c in range(NCH):
            tsub = tmp_p.tile([128, B_CORE], BF16, tag="flt")
            nc.vector.tensor_sub(tsub, xbs[c], mRb)
            xc = tmp_p.tile([128, B_CORE], BF16, tag="fxn", name=f"fxn{c}", bufs=6)
            nc.vector.tensor_tensor(out=xc, in0=tsub, in1=rRb, op=ALU.mult)
            xnf.append(xc)
        for mc in range(8):
            ps = pslot()
            psv = ps[:, 0:B_CORE]
            for kc in range(NCH):
                nc.tensor.matmul(psv, hw_sb[:, kc, mc * 128:(mc + 1) * 128], xnf[kc],
                                 start=(kc == 0), stop=(kc == NCH - 1))
            ot = tmp_p.tile([128, B_CORE], F32, tag="fout")
            nc.scalar.activation(out=ot, in_=psv, func=AF.Identity,
                                 bias=hb_sb[:, mc:mc + 1], scale=1.0)
            nc.sync.dma_start(out=OUT[mc * 128:(mc + 1) * 128, :], in_=ot)

    nc.compile()
    return nc


# ---------------------------------------------------------------------------
# host side
# ---------------------------------------------------------------------------

def _rel_nkm():
    ind = np.arange(GRID)[None, :] - np.arange(GRID)[:, None]
    indx = np.tile(ind, (GRID, GRID)).astype(np.float32)
    indy = np.repeat(np.repeat(ind, GRID, axis=0), GRID, axis=1).astype(np.float32)
    indd = indx ** 2 + indy ** 2
    rel = np.stack([indx, indy, indd], axis=0)           # [3, n, m]
    return np.ascontiguousarray(rel.transpose(1, 0, 2))  # [n, 3, m]


def _pcol(v, parts=128):
    v = np.asarray(v, np.float32).reshape(-1, parts)
    return np.ascontiguousarray(v.T)


def _wT(w):
    return np.ascontiguousarray(np.asarray(w, np.float32).T.astype(BF))


def _prep_weights(i):
    d = {}
    d["pwT"] = _wT(np.asarray(i["patch_w"], np.float32).reshape(C, C))
    # pos_embed with patch_b folded in
    posT = np.asarray(i["pos_embed"], np.float32)[0].T.copy()   # [C, N]
    posT += np.asarray(i["patch_b"], np.float32)[:, None]
    d["posT"] = np.ascontiguousarray(posT)
    d["cls"] = np.asarray(i["cls_token"], np.float32).reshape(C)
    ns = np.asarray(i["norm_s"], np.float32)
    nb = np.asarray(i["norm_b"], np.float32)
    hw = np.asarray(i["head_w"], np.float32)               # [1000, C]
    hT = np.zeros((C, 1024), np.float32)
    hT[:, :1000] = (hw * ns[None, :]).T
    d["headT"] = np.ascontiguousarray(hT.astype(BF))
    hb = np.zeros(1024, np.float32)
    hb[:1000] = np.asarray(i["head_b"], np.float32) + hw @ nb
    d["head_b"] = _pcol(hb)
    for l in range(L_G):
        s1 = np.asarray(i["g_norm1_s"][l], np.float32)
        b1 = np.asarray(i["g_norm1_b"][l], np.float32)
        s2 = np.asarray(i["g_norm2_s"][l], np.float32)
        b2 = np.asarray(i["g_norm2_b"][l], np.float32)
        qk = np.asarray(i["g_qk_w"][l], np.float32)        # [2C, C]
        vw = np.asarray(i["g_v_w"][l], np.float32)         # [C, C]
        pw = np.asarray(i["g_proj_w"][l], np.float32)      # [C, C]
        fc1 = np.asarray(i["g_fc1_w"][l], np.float32)      # [4C, C]
        d[f"g{l}_qkT"] = _wT(qk * s1[None, :])
        d[f"g{l}_qkb"] = _pcol(qk @ b1)
        d[f"g{l}_vT"] = _wT(vw * s1[None, :])
        d[f"g{l}_projT"] = _wT(pw)
        d[f"g{l}_projb"] = _pcol(np.asarray(i["g_proj_b"][l], np.float32) + pw @ (vw @ b1))
        d[f"g{l}_fc1T"] = _wT(fc1 * s2[None, :])
        d[f"g{l}_fc1b"] = _pcol(np.asarray(i["g_fc1_b"][l], np.float32) + fc1 @ b2)
        d[f"g{l}_fc2T"] = _wT(i["g_fc2_w"][l])
        d[f"g{l}_fc2b"] = _pcol(i["g_fc2_b"][l])
        # pos-softmax (weight-only): scores[h,n,m] = sum_k rel[n,k,m]*posw[h,k]
        relnkm = _rel_nkm()
        posw = np.asarray(i["g_pos_w"][l], np.float32)
        sc = np.einsum('nkm,hk->hnm', relnkm, posw)
        sc -= sc.max(axis=-1, keepdims=True)
        e = np.exp(sc)
        pos = e / e.sum(axis=-1, keepdims=True)            # [H, N, M]
        ftT = np.ascontiguousarray(pos.transpose(2, 0, 1).reshape(NPATCH, H * NPATCH)
                                   .astype(BF))            # [M, H*N]
        d[f"g{l}_ft"] = ftT
        sig = 1.0 / (1.0 + np.exp(-np.asarray(i["g_gate"][l], np.float32)))
        d[f"g{l}_omdiv"] = np.ascontiguousarray(
            np.tile(np.repeat(1.0 / (1.0 - sig), 64).reshape(1, H * 64), (128, 1)).astype(BF))
        d[f"g{l}_g64"] = np.ascontiguousarray(np.tile(sig.reshape(1, H), (64, 1)))
    for l in range(L_M):
        s1 = np.asarray(i["m_norm1_s"][l], np.float32)
        b1 = np.asarray(i["m_norm1_b"][l], np.float32)
        s2 = np.asarray(i["m_norm2_s"][l], np.float32)
        b2 = np.asarray(i["m_norm2_b"][l], np.float32)
        qkv = np.asarray(i["m_qkv_w"][l], np.float32)      # [3C, C]
        pw = np.asarray(i["m_proj_w"][l], np.float32)
        fc1 = np.asarray(i["m_fc1_w"][l], np.float32)
        d[f"m{l}_qkvT"] = _wT(qkv * s1[None, :])
        d[f"m{l}_qkb"] = _pcol(qkv[:2 * C] @ b1)
        d[f"m{l}_projT"] = _wT(pw)
        d[f"m{l}_projb"] = _pcol(np.asarray(i["m_proj_b"][l], np.float32)
                                 + pw @ (qkv[2 * C:] @ b1))
        d[f"m{l}_fc1T"] = _wT(fc1 * s2[None, :])
        d[f"m{l}_fc1b"] = _pcol(np.asarray(i["m_fc1_b"][l], np.float32) + fc1 @ b2)
        d[f"m{l}_fc2T"] = _wT(i["m_fc2_w"][l])
        d[f"m{l}_fc2b"] = _pcol(i["m_fc2_b"][l])
    return d


_last_results = None


def build_in_maps(inputs):
    wmap = _prep_weights(inputs)
    x = np.asarray(inputs["x"], np.float32)
    in_maps = []
    for core in range(NCORES):
        xs = x[core * B_CORE:(core + 1) * B_CORE]
        xi = xs.reshape(B_CORE, 3, GRID, PS, GRID, PS).transpose(1, 3, 5, 0, 2, 4)
        xi = np.ascontiguousarray(xi.reshape(C, B_CORE * NPATCH).astype(BF))
        m = dict(wmap)
        m["xim"] = xi
        in_maps.append(m)
    return in_maps


def get_program():
    if "nc" not in _CACHE:
        _CACHE["nc"] = _build_program()
    return _CACHE["nc"]


def kernel(**inputs):
    global _last_results
    _install_ntff_hook()
    from concourse import bass_utils

    nc = get_program()
    in_maps = build_in_maps(inputs)
    res = bass_utils.run_bass_kernel_spmd(nc, in_maps, core_ids=list(range(NCORES)))
    _last_results = res
    outs = [r["out"][:1000, :].T for r in res.results]
    return np.ascontiguousarray(np.concatenate(outs, axis=0).astype(np.float32))



# revision 38
# speedup vs baseline: 1.1458x; 1.1432x over previous
"""ConViT (12-layer, H=12, D=64, B=64) forward pass on 8 TRN2 NeuronCores.

Strategy: data-parallel over batch (8 images per core). Host does layout prep
only (im2col of the non-overlapping patch conv, weight transposes to [ci, co],
bf16 casts, LayerNorm affine folding into consumer weights); all FLOPs run
on-device. Activations are feature-major [C -> 6x128 partitions, tokens free];
matmuls are bf16 with f32 PSUM; residual stream in f32.

Perf notes vs v1:
 - LN affine (s, b) folded into qk/v/fc1/head weights on host; V-path bias
   eliminated entirely (softmax rows sum to 1) and absorbed into proj bias.
 - proj/fc2 bias folded into the PSUM->residual scalar_tensor_tensor epilogue.
 - reciprocal_approx_fast instead of iterative reciprocal.
 - single persistent PSUM ring pool (no per-phase pool barriers).
 - attention: merged exp over both key halves; softmax denominator broadcast
   into rows 64:128 of the same PSUM slot (col-tiled matmul); one-head
   lookahead pipeline.
"""
import os
import sys
import types
import contextlib
import ctypes
from contextlib import ExitStack

import numpy as np
import ml_dtypes

import concourse.bass as bass
import concourse.mybir as mybir
import concourse.tile as tile
from concourse import bacc
from concourse.masks import make_identity

F32 = mybir.dt.float32
BF16 = mybir.dt.bfloat16
AF = mybir.ActivationFunctionType
ALU = mybir.AluOpType
BF = ml_dtypes.bfloat16

H = 12
D = 64
C = 768
NCH = 6             # C / 128
PS = 16             # patch size
GRID = 14
NPATCH = 196        # tokens per image in GPSA phase
NTOK = 197          # tokens per image in MHSA phase (cls + patches)
L_G = 3
L_M = 9
SCALE = D ** -0.5
EPS = 1e-5
B_CORE = 8
NCORES = 8
MLPC = 24           # 3072 / 128

_CACHE = {}
FAST_RECIP = False


def _install_ntff_hook():
    """Best-effort: enable NTFF profiling under axon (used when BASS_TRACE=1)."""
    if "antenv.axon_hooks" in sys.modules:
        return
    so_path = "/opt/axon/libaxon_pjrt.so"
    if not os.path.exists(so_path):
        return
    try:
        lib = ctypes.CDLL(so_path)
        if not hasattr(lib, "axon_start_nrt_profile"):
            return
        lib.axon_start_nrt_profile.argtypes = [ctypes.POINTER(ctypes.c_int64), ctypes.c_size_t]
        lib.axon_start_nrt_profile.restype = ctypes.c_int64
        lib.axon_stop_nrt_profile.argtypes = [ctypes.c_char_p]
        lib.axon_stop_nrt_profile.restype = ctypes.c_int64

        @contextlib.contextmanager
        def _hook(output_dir, device_ids):
            import jax
            jax.devices()
            if device_ids:
                ids = (ctypes.c_int64 * len(device_ids))(*device_ids)
                rc = lib.axon_start_nrt_profile(ids, len(device_ids))
            else:
                rc = lib.axon_start_nrt_profile(None, 0)
            if rc != 0:
                raise RuntimeError(f"axon_start_nrt_profile rc={rc}")
            try:
                yield
            finally:
                n = lib.axon_stop_nrt_profile(str(output_dir).encode())
                if n < 0:
                    raise RuntimeError(f"axon_stop_nrt_profile rc={n}")

        mod = types.ModuleType("antenv.axon_hooks")
        mod._hook = _hook
        mod.get_axon_ntff_profile_hook = lambda: mod._hook
        mod.set_axon_ntff_profile_hook = lambda h: setattr(mod, "_hook", h)
        sys.modules["antenv.axon_hooks"] = mod
        import antenv
        antenv.axon_hooks = mod
    except Exception:
        pass


def _build_program():
    nc = bacc.Bacc("TRN2", target_bir_lowering=False, debug=False)

    def din(name, shape, dt):
        return nc.dram_tensor(name, shape, dt, kind="ExternalInput").ap()

    xim = din("xim", [C, B_CORE * NPATCH], BF16)
    pwT = din("pwT", [C, C], BF16)
    posT = din("posT", [C, NPATCH], F32)           # pos_embed + patch_b folded
    cls = din("cls", [C], F32)
    headT = din("headT", [C, 1024], BF16)          # * norm_s folded
    head_b = din("head_b", [128, 8], F32)          # + headT @ norm_b folded

    gl, ml = [], []
    for i in range(L_G):
        gl.append({
            "qkT": din(f"g{i}_qkT", [C, 2 * C], BF16),      # * n1s folded
            "qkb": din(f"g{i}_qkb", [128, 2 * NCH], F32),   # qk_w @ n1b
            "vT": din(f"g{i}_vT", [C, C], BF16),            # * n1s folded
            "projT": din(f"g{i}_projT", [C, C], BF16),
            "projb": din(f"g{i}_projb", [128, NCH], F32),   # + proj_w @ (v_w @ n1b)
            "fc1T": din(f"g{i}_fc1T", [C, 4 * C], BF16),    # * n2s folded
            "fc1b": din(f"g{i}_fc1b", [128, MLPC], F32),    # + fc1_w @ n2b
            "fc2T": din(f"g{i}_fc2T", [4 * C, C], BF16),
            "fc2b": din(f"g{i}_fc2b", [128, NCH], F32),
            "ft": din(f"g{i}_ft", [NPATCH, H * NPATCH], BF16),  # pos-softmax^T
            "omdiv": din(f"g{i}_omdiv", [128, H * 64], BF16),   # 1/(1-sigmoid(gate))
            "g64": din(f"g{i}_g64", [64, H], F32),              # sigmoid(gate) bcast
        })
    for i in range(L_M):
        ml.append({
            "qkvT": din(f"m{i}_qkvT", [C, 3 * C], BF16),    # * n1s folded
            "qkb": din(f"m{i}_qkb", [128, 2 * NCH], F32),   # qk rows of qkv_w @ n1b
            "projT": din(f"m{i}_projT", [C, C], BF16),
            "projb": din(f"m{i}_projb", [128, NCH], F32),   # + proj_w @ (v_w @ n1b)
            "fc1T": din(f"m{i}_fc1T", [C, 4 * C], BF16),
            "fc1b": din(f"m{i}_fc1b", [128, MLPC], F32),
            "fc2T": din(f"m{i}_fc2T", [4 * C, C], BF16),
            "fc2b": din(f"m{i}_fc2b", [128, NCH], F32),
        })

    OUT = nc.dram_tensor("out", [1024, B_CORE], F32, kind="ExternalOutput").ap()

    MIDTAGS = [f"qt{j}" for j in range(NCH)] + [f"kt{j}" for j in range(NCH)]

    with ExitStack() as ctx:
        tc = ctx.enter_context(tile.TileContext(nc))

        consts = ctx.enter_context(tc.tile_pool(name="consts", bufs=1))
        res_p = ctx.enter_context(tc.tile_pool(name="res", bufs=1))
        act_p = ctx.enter_context(tc.tile_pool(name="act", bufs=2))   # xn / OT / xn2
        qk_p = ctx.enter_context(tc.tile_pool(name="qk", bufs=2))     # Q^T/K^T + mids + xb
        v_p = ctx.enter_context(tc.tile_pool(name="vp", bufs=1))      # token-major V
        w_p = ctx.enter_context(tc.tile_pool(name="wp", bufs=1))      # streamed weights
        wsm_p = ctx.enter_context(tc.tile_pool(name="wsm", bufs=2))   # per-layer params
        row_p = ctx.enter_context(tc.tile_pool(name="rows", bufs=2))  # [1, *] stat rows
        rowa_p = ctx.enter_context(tc.tile_pool(name="rowsa", bufs=2))  # attn recip rows
        mrb_p = ctx.enter_context(tc.tile_pool(name="mrb", bufs=2))   # bcast mean/rstd bf16
        tmp_p = ctx.enter_context(tc.tile_pool(name="tmp", bufs=2))   # scratch tiles
        tmp1_p = ctx.enter_context(tc.tile_pool(name="tmp1", bufs=2))  # big f32 scratch
        e_p = ctx.enter_context(tc.tile_pool(name="ep", bufs=4))      # exp tiles
        ft_p = ctx.enter_context(tc.tile_pool(name="ft", bufs=1))     # GPSA pos F^T

        # single persistent PSUM pool: "ps" ring of 6 one-bank slots + "mmv"
        psum = ctx.enter_context(tc.tile_pool(name="psum", bufs=6, space="PSUM"))

        def pslot():
            # exactly one 2KB PSUM bank -> slots stay bank-aligned
            return psum.tile([128, 512], F32, tag="ps", name="ps")

        def pv(t):
            # [128, 2, NTOK] view of a bank slot
            return t[:, 0:2 * NTOK].rearrange("p (b t) -> p b t", t=NTOK)

        def recip(out, in_):
            if FAST_RECIP:
                nc.vector.reciprocal_approx_fast(out=out, in_=in_)
            else:
                nc.vector.reciprocal(out=out, in_=in_)

        def _scalar_act_raw(out, in_, func, bias):
            # InstActivation without bass's Reciprocal/Rsqrt accuracy guard.
            eng = nc.scalar
            ins = [eng.lower_ap(in_)]
            for arg in (bias, 1.0, 0.0):  # bias, scale, alpha
                if isinstance(arg, bass.AP):
                    ins.append(eng.lower_ap(arg))
                else:
                    ins.append(mybir.ImmediateValue(dtype=F32, value=arg))
            return eng.add_instruction(mybir.InstActivation(
                name=eng.bass.get_next_instruction_name(),
                func=func, ins=ins, outs=[eng.lower_ap(out)]))

        def scalar_recip(out, in_):
            return _scalar_act_raw(out, in_, AF.Reciprocal, 0.0)

        def scalar_rsqrt(out, in_, bias_ap):
            return _scalar_act_raw(out, in_, AF.Rsqrt, bias_ap)

        ones_col = consts.tile([128, 1], BF16)
        nc.vector.memset(ones_col, 1.0)
        ones_row = consts.tile([1, 128], BF16)
        nc.vector.memset(ones_row, 1.0)
        ones64 = consts.tile([128, 64], BF16)
        nc.vector.memset(ones64, 1.0)
        eps_sb = consts.tile([128, 1], F32)
        nc.vector.memset(eps_sb, EPS)

        res = [res_p.tile([128, B_CORE, NTOK], F32, tag=f"res{c}", name=f"res{c}") for c in range(NCH)]

        # persistent token-major V tiles
        vtiles = {}
        for bi in range(2):
            for hi in range(2):
                vsb = v_p.tile([128, H, 64], BF16, tag=f"v{bi}{hi}", name=f"v{bi}{hi}")
                vtiles[(bi, hi)] = vsb

        # cls token into res[:, :, 0]
        for c in range(NCH):
            src = cls[c * 128:(c + 1) * 128]
            ap = bass.AP(tensor=src.tensor, offset=src.offset,
                         ap=[list(src.ap[0]), [0, B_CORE], [0, 1]])
            nc.sync.dma_start(out=res[c][:, :, 0:1], in_=ap)

        def load_wT(dram, ncol, tag):
            t = w_p.tile([128, NCH, ncol], BF16, tag=tag)
            nc.sync.dma_start(out=t, in_=dram.rearrange("(c p) n -> p c n", p=128))
            return t

        def load_sm(dram, ncol, tag, dt=F32):
            t = wsm_p.tile([128, ncol], dt, tag=tag)
            nc.sync.dma_start(out=t, in_=dram)
            return t

        # ---- patch embed -------------------------------------------------
        pw_sb = load_wT(pwT, C, "wbig")
        xim_sb = w_p.tile([128, MLPC, C], BF16, tag="wbig2")  # reuse fc2-size slot
        ximv = xim_sb.rearrange("p a b -> p (a b)")[:, 0:NCH * B_CORE * NPATCH] \
            .rearrange("p (c n) -> p c n", c=NCH)
        ximd = xim.rearrange("(c p) n -> p c n", p=128)
        for nv in range(4):
            sl = slice(2 * nv * NPATCH, (2 * nv + 2) * NPATCH)
            nc.sync.dma_start(out=ximv[:, :, sl], in_=ximd[:, :, sl])

        for nv in range(4):
            b0 = 2 * nv
            for mc in range(NCH):
                ps = pslot()
                psf = ps[:, 0:2 * NPATCH]
                for kc in range(NCH):
                    nc.tensor.matmul(
                        psf, pw_sb[:, kc, mc * 128:(mc + 1) * 128],
                        ximv[:, kc, b0 * NPATCH:(b0 + 2) * NPATCH],
                        start=(kc == 0), stop=(kc == NCH - 1))
                nc.scalar.activation(
                    out=res[mc][:, b0:b0 + 2, 1:NTOK],
                    in_=psf.rearrange("p (b t) -> p b t", b=2),
                    func=AF.Identity, scale=1.0)
        for c in range(NCH):
            src = posT[c * 128:(c + 1) * 128]
            ap = bass.AP(tensor=src.tensor, offset=src.offset,
                         ap=[list(src.ap[0]), [0, B_CORE], list(src.ap[1])])
            nc.gpsimd.dma_start(out=res[c][:, :, 1:NTOK], in_=ap, accum_op=ALU.add)

        # ---- helpers -----------------------------------------------------
        def make_ln(xn, t0, tl):
            """res -> xn bf16 normalized (no affine; folded into consumers).
            Returns (ls, bn): ls(pair) emits stats + row chains for image
            pairs 2p,2p+1; bn(pair) emits broadcast + normalize. Callers
            weave these between dense phases so the serial row chain hides
            behind matmul work."""
            ntl = 2 * tl
            chains = {}

            def stats(nv):
                b0 = 2 * nv
                s_ps = pslot()
                q_ps = pslot()
                s_row = s_ps[0:1, 0:ntl]
                q_row = q_ps[0:1, 0:ntl]
                for c in range(NCH):
                    xb = qk_p.tile([128, 2, tl], BF16, tag=MIDTAGS[c], name=f"xb{c}")
                    xq = tmp_p.tile([128, 2, tl], BF16, tag="xq")
                    sl = res[c][:, b0:b0 + 2, t0:t0 + tl]
                    nc.scalar.activation(out=xb, in_=sl, func=AF.Identity)
                    nc.vector.tensor_tensor(out=xq, in0=xb, in1=xb, op=ALU.mult)
                    nc.tensor.matmul(s_row, ones_col, xb.rearrange("p b t -> p (b t)"),
                                     start=(c == 0), stop=(c == NCH - 1))
                    nc.tensor.matmul(q_row, ones_col, xq.rearrange("p b t -> p (b t)"),
                                     start=(c == 0), stop=(c == NCH - 1))
                return s_row, q_row

            def rowchain(s_row, q_row):
                v1 = row_p.tile([1, ntl], F32, tag="v1")
                nc.vector.tensor_scalar_mul(v1, s_row, 1.0 / C)
                meanb = row_p.tile([1, ntl], BF16, tag="meanb")
                nc.scalar.activation(out=meanb, in_=v1, func=AF.Identity)
                nc.vector.tensor_tensor(out=v1, in0=v1, in1=v1, op=ALU.mult)
                nc.vector.scalar_tensor_tensor(out=v1, in0=q_row, scalar=1.0 / C,
                                               in1=v1, op0=ALU.mult, op1=ALU.subtract)
                nc.scalar.activation(out=v1, in_=v1, func=AF.Sqrt,
                                     bias=eps_sb[0:1, :], scale=1.0)
                nc.vector.reciprocal_approx_fast(out=v1, in_=v1)
                rstdb = row_p.tile([1, ntl], BF16, tag="rstdb")
                nc.scalar.activation(out=rstdb, in_=v1, func=AF.Identity)
                return meanb, rstdb

            def bcast_norm(nv, meanb, stdb):
                b0 = 2 * nv
                mR = pslot()
                rR = pslot()
                mRf = mR[:, 0:ntl]
                rRf = rR[:, 0:ntl]
                nc.tensor.matmul(mRf, ones_row, meanb, start=True, stop=True)
                nc.tensor.matmul(rRf, ones_row, stdb, start=True, stop=True)
                mRb = mrb_p.tile([128, 2, tl], BF16, tag="mRb")
                rRb = mrb_p.tile([128, 2, tl], BF16, tag="rRb")
                nc.scalar.activation(out=mRb.rearrange("p b t -> p (b t)"), in_=mRf,
                                     func=AF.Identity)
                nc.scalar.activation(out=rRb.rearrange("p b t -> p (b t)"), in_=rRf,
                                     func=AF.Identity)
                for c in range(NCH):
                    tsub = tmp_p.tile([128, 2, tl], BF16, tag="lnt")
                    nc.vector.tensor_sub(tsub, res[c][:, b0:b0 + 2, t0:t0 + tl], mRb)
                    nc.vector.tensor_tensor(out=xn[c][:, b0:b0 + 2, t0:t0 + tl],
                                            in0=tsub, in1=rRb, op=ALU.mult)

            def ls(pair):
                st = [stats(2 * pair + i) for i in range(2)]
                chains[pair] = [rowchain(s, q) for s, q in st]

            def bn(nv):
                bcast_norm(nv, *chains[nv // 2][nv % 2])

            return ls, bn

        def make_proj(wT_sb, biast, t0, tl, rhs_of):
            """proj(nv): res += (rhs @ W^T) + bias."""
            def proj(nv):
                b0 = 2 * nv
                for mc in range(NCH):
                    ps = pslot()
                    psf = ps[:, 0:2 * tl]
                    for kc in range(NCH):
                        nc.tensor.matmul(
                            psf, wT_sb[:, kc, mc * 128:(mc + 1) * 128], rhs_of(kc, b0),
                            start=(kc == 0), stop=(kc == NCH - 1))
                    sl = res[mc][:, b0:b0 + 2, t0:t0 + tl]
                    nc.vector.scalar_tensor_tensor(
                        out=sl, in0=psf.rearrange("p (b t) -> p b t", b=2),
                        scalar=biast[:, mc:mc + 1], in1=sl,
                        op0=ALU.add, op1=ALU.add)
            return proj

        def make_mlp(L, xn, t0, tl):
            fc1_sb = load_wT(L["fc1T"], 4 * C, "wbig")
            fc1b_sb = load_sm(L["fc1b"], MLPC, "fc1b")
            fc2_sb = w_p.tile([128, MLPC, C], BF16, tag="wbig2")
            nc.sync.dma_start(out=fc2_sb, in_=L["fc2T"].rearrange("(c p) n -> p c n", p=128))
            fc2b_sb = load_sm(L["fc2b"], NCH, "fc2b")

            def mlp_nv(nv):
                b0 = 2 * nv
                mids = []
                for mc in range(MLPC):
                    ps = pslot()
                    psf = ps[:, 0:2 * tl]
                    for kc in range(NCH):
                        nc.tensor.matmul(
                            psf, fc1_sb[:, kc, mc * 128:(mc + 1) * 128],
                            xn[kc][:, b0:b0 + 2, t0:t0 + tl],
                            start=(kc == 0), stop=(kc == NCH - 1))
                    mt = qk_p.tile([128, 2 * NTOK], BF16, tag=MIDTAGS[mc % 12])
                    nc.scalar.activation(out=mt[:, 0:2 * tl], in_=psf, func=AF.Gelu,
                                         bias=fc1b_sb[:, mc:mc + 1], scale=1.0)
                    mids.append(mt)
                for mc in range(NCH):
                    ps = pslot()
                    psf = ps[:, 0:2 * tl]
                    for kc in range(MLPC):
                        nc.tensor.matmul(
                            psf, fc2_sb[:, kc, mc * 128:(mc + 1) * 128],
                            mids[kc][:, 0:2 * tl],
                            start=(kc == 0), stop=(kc == MLPC - 1))
                    sl = res[mc][:, b0:b0 + 2, t0:t0 + tl]
                    nc.vector.scalar_tensor_tensor(
                        out=sl, in0=psf.rearrange("p (b t) -> p b t", b=2),
                        scalar=fc2b_sb[:, mc:mc + 1], in1=sl,
                        op0=ALU.add, op1=ALU.add)
            return mlp_nv

        def make_qa(L, xn, OT, t0, tl, gpsa, pos_ctx):
            """qa(nv): QKV projections -> V build -> attention -> OT."""
            kl = tl - 128
            nkeys = [(0, 128), (128, kl)]
            w_qk = pos_ctx["w_qk"]
            w_v = pos_ctx["w_v"]
            qkb_sb = pos_ctx["qkb"]

            def qa(nv):
                b0 = 2 * nv
                qt = [qk_p.tile([128, 2, NTOK], BF16, tag=f"qt{c}", name=f"qtt{c}") for c in range(NCH)]
                kt = [qk_p.tile([128, 2, NTOK], BF16, tag=f"kt{c}", name=f"ktt{c}") for c in range(NCH)]
                for mc in range(2 * NCH):
                    ps = pslot()
                    psf = ps[:, 0:2 * tl]
                    for kc in range(NCH):
                        nc.tensor.matmul(
                            psf, w_qk[:, kc, mc * 128:(mc + 1) * 128],
                            xn[kc][:, b0:b0 + 2, t0:t0 + tl],
                            start=(kc == 0), stop=(kc == NCH - 1))
                    dst = qt[mc] if mc < NCH else kt[mc - NCH]
                    nc.scalar.activation(
                        out=dst[:, :, 0:tl],
                        in_=psf.rearrange("p (b t) -> p b t", b=2), func=AF.Identity,
                        bias=qkb_sb[:, mc:mc + 1], scale=1.0)
                vt = vtiles
                for bi in range(2):
                    b = b0 + bi
                    for hi, (h0, hl) in enumerate(nkeys):
                        vsb = vt[(bi, hi)]
                        ps = psum.tile([128, C], F32, tag="mmv", bufs=1, name="psv",
                                       padded_shape=[128, 1024])
                        for kc in range(NCH):
                            for c0, cl in ((0, 512), (512, 256)):
                                nc.tensor.matmul(
                                    ps[:hl, c0:c0 + cl],
                                    xn[kc][:, b, t0 + h0:t0 + h0 + hl],
                                    w_v(kc)[:, c0:c0 + cl],
                                    start=(kc == 0), stop=(kc == NCH - 1))
                        nc.scalar.activation(
                            out=vsb[:hl, :, 0:64],
                            in_=ps[:hl].rearrange("p (h d) -> p h d", h=H),
                            func=AF.Identity)

                def avden(h, es):
                    """AV matmuls + ones/omdiv denominator broadcast."""
                    oe = pv(pslot())
                    for bi in range(2):
                        nc.tensor.matmul(oe[0:64, bi, 0:tl], vt[(bi, 0)][:, h, 0:64],
                                         es[bi][:, 0, 0:tl], start=(bi == 0), stop=False)
                        nc.tensor.matmul(oe[0:64, bi, 0:tl], vt[(bi, 1)][:kl, h, 0:64],
                                         es[bi][:kl, 1, 0:tl], start=False, stop=(bi == 1))
                    db = pv(pslot())
                    dlhs = pos_ctx["omdiv"] if gpsa else None
                    for bi in range(2):
                        nc.tensor.matmul(db[0:64, bi, 0:tl],
                                         dlhs[:, h, :] if gpsa else ones64,
                                         es[bi][:, 0, 0:tl], start=(bi == 0), stop=False)
                        nc.tensor.matmul(db[0:64, bi, 0:tl],
                                         dlhs[0:kl, h, :] if gpsa else ones64[0:kl],
                                         es[bi][:kl, 1, 0:tl], start=False, stop=(bi == 1))
                    return oe, db

                def normalize(h, oedb, fp):
                    oe, db = oedb
                    ch, off = h // 2, (h % 2) * 64
                    r_sb = rowa_p.tile([64, 2, NTOK], F32, tag="db", bufs=2)
                    nc.vector.reciprocal_approx_fast(out=r_sb[:, :, 0:tl],
                                                     in_=db[0:64, :, 0:tl])
                    for bi in range(2):
                        b = b0 + bi
                        if gpsa:
                            tf = tmp1_p.tile([64, NTOK], BF16, tag="tf")
                            nc.vector.tensor_tensor(out=tf[:, 0:tl],
                                                    in0=oe[0:64, bi, 0:tl],
                                                    in1=r_sb[:, bi, 0:tl], op=ALU.mult)
                            nc.vector.scalar_tensor_tensor(
                                out=OT[ch][off:off + 64, b, t0:t0 + tl],
                                in0=fp[0:64, bi, 0:tl], scalar=pos_ctx["g64"][:, h:h + 1],
                                in1=tf[:, 0:tl], op0=ALU.mult, op1=ALU.add)
                        else:
                            nc.vector.tensor_tensor(
                                out=OT[ch][off:off + 64, b, t0:t0 + tl],
                                in0=oe[0:64, bi, 0:tl], in1=r_sb[:, bi, 0:tl],
                                op=ALU.mult)

                if gpsa:
                    def stageA(h):
                        ch, off = h // 2, (h % 2) * 64
                        es = []
                        for bi in range(2):
                            s = pv(pslot())
                            nc.tensor.matmul(s[:, 0, 0:tl],
                                             kt[ch][off:off + 64, bi, 0:128],
                                             qt[ch][off:off + 64, bi, 0:tl],
                                             start=True, stop=True)
                            nc.tensor.matmul(s[:kl, 1, 0:tl],
                                             kt[ch][off:off + 64, bi, 128:tl],
                                             qt[ch][off:off + 64, bi, 0:tl],
                                             start=True, stop=True)
                            e = e_p.tile([128, 2, NTOK], BF16, tag="e", name="e",
                                         bufs=8)
                            # rows kl:128 of the second half are stale garbage;
                            # never read downstream.
                            nc.scalar.activation(out=e[:, :, 0:tl], in_=s[:, :, 0:tl],
                                                 func=AF.Exp, scale=SCALE)
                            es.append(e)
                        return es

                    def stageB(h, es):
                        FT = pos_ctx["FT"]
                        fp = pv(pslot())
                        for bi in range(2):
                            nc.tensor.matmul(fp[0:64, bi, 0:tl], vt[(bi, 0)][:, h, 0:64],
                                             FT[0][:, h, :], start=True, stop=False)
                            nc.tensor.matmul(fp[0:64, bi, 0:tl], vt[(bi, 1)][:kl, h, 0:64],
                                             FT[1][:kl, h, :], start=False, stop=True)
                        oe = avden(h, es)
                        normalize(h, oe, fp)

                    prev = None
                    for h in range(H):
                        es = stageA(h)
                        if prev is not None:
                            stageB(*prev)
                        prev = (h, es)
                    stageB(*prev)
                else:
                    # MHSA: head pairs (2j, 2j+1) live at partition offsets 0/64
                    # of chunk j -> row-tiled score matmuls run concurrently.
                    def stageA(j):
                        ss = {}
                        for idx in range(2):
                            for bi in range(2):
                                ss[(idx, bi)] = pv(pslot())
                        for bi in range(2):
                            for ci, (c0, cl_) in enumerate(((0, 128), (128, kl))):
                                for idx, off in ((0, 0), (1, 64)):
                                    s = ss[(idx, bi)]
                                    nc.tensor.matmul(
                                        s[0:cl_, ci, 0:tl],
                                        kt[j][off:off + 64, bi, c0:c0 + cl_],
                                        qt[j][off:off + 64, bi, 0:tl],
                                        start=True, stop=True)
                        es = {}
                        for idx in range(2):
                            for bi in range(2):
                                e = e_p.tile([128, 2, NTOK], BF16, tag="e", name="e",
                                             bufs=8)
                                nc.scalar.activation(out=e[:, :, 0:tl],
                                                     in_=ss[(idx, bi)][:, :, 0:tl],
                                                     func=AF.Exp, scale=SCALE)
                                es[(idx, bi)] = e
                        return es

                    def stageB(j, es):
                        for idx in range(2):
                            h = 2 * j + idx
                            epair = [es[(idx, 0)], es[(idx, 1)]]
                            oe = avden(h, epair)
                            normalize(h, oe, None)

                    prev = None
                    for j in range(H // 2):
                        es = stageA(j)
                        if prev is not None:
                            stageB(*prev)
                        prev = (j, es)
                    stageB(*prev)

            return qa

        def make_ln1(gpsa):
            t0, tl = (1, NPATCH) if gpsa else (0, NTOK)
            xn = [act_p.tile([128, B_CORE, NTOK], BF16, tag=f"act{c}", name=f"xn{c}")
                  for c in range(NCH)]
            ls1, bn1 = make_ln(xn, t0, tl)
            return xn, ls1, bn1

        def emit_layer(L, gpsa, pre_ln1, next_gpsa):
            """Emit one transformer layer, weaving LN stages between dense
            phases so their serial row chains hide behind PE matmul work.
            pre_ln1: (xn, ls1, bn1) with ls1(0) and bn1(0) already emitted
            by the previous layer's tail; bn1(1..3)/ls1(1) still pending.
            next_gpsa: None at the last layer, else next layer's gpsa flag;
            returns next layer's pre_ln1."""
            t0, tl = (1, NPATCH) if gpsa else (0, NTOK)
            projb_sb = load_sm(L["projb"], NCH, "projb")
            qkb_sb = load_sm(L["qkb"], 2 * NCH, "qkb")

            pos_ctx = {"qkb": qkb_sb}
            if gpsa:
                pos_ctx["w_qk"] = load_wT(L["qkT"], 2 * C, "wbig")
                v_sb = load_wT(L["vT"], C, "wbig2")
                pos_ctx["w_v"] = lambda kc: v_sb[:, kc, :]
                # host-precomputed pos-softmax^T [key m, head, query n]
                FT = [ft_p.tile([128, H, NPATCH], BF16, tag=f"ft{i}", name=f"ft{i}") for i in range(2)]
                pos_ctx["FT"] = FT
                ftd = L["ft"].rearrange("m (h n) -> m h n", h=H)
                nc.sync.dma_start(out=FT[0][:128], in_=ftd[0:128])
                nc.sync.dma_start(out=FT[1][:68], in_=ftd[128:196])
                omdiv = wsm_p.tile([128, H, 64], BF16, tag="omdiv")
                nc.sync.dma_start(out=omdiv, in_=L["omdiv"].rearrange("p (h d) -> p h d", h=H))
                pos_ctx["omdiv"] = omdiv
                g64 = wsm_p.tile([64, H], F32, tag="g64")
                nc.sync.dma_start(out=g64, in_=L["g64"])
                pos_ctx["g64"] = g64
            else:
                qkv_sb = load_wT(L["qkvT"], 3 * C, "wbig")
                pos_ctx["w_qk"] = qkv_sb
                pos_ctx["w_v"] = lambda kc: qkv_sb[:, kc, 2 * C:3 * C]

            if pre_ln1 is None:
                xn, ls1, bn1 = make_ln1(gpsa)
                ls1(0)
                bn1(0)
                bn1(1)
                ls1(1)
                bn1(2)
                bn1(3)
            else:
                xn, ls1, bn1 = pre_ln1

            OT = [act_p.tile([128, B_CORE, NTOK], BF16, tag=f"act{c}", name=f"ot{c}") for c in range(NCH)]
            qa = make_qa(L, xn, OT, t0, tl, gpsa, pos_ctx)

            def mkprj():
                proj_sb = load_wT(L["projT"], C, "wbig2")
                return make_proj(proj_sb, projb_sb, t0, tl,
                                 lambda kc, b0: OT[kc][:, b0:b0 + 2, t0:t0 + tl])

            qa(0)
            if pre_ln1 is not None:
                ls1(1)
            if not gpsa:
                # proj weights go to wbig2 (free since last layer's fc2)
                prj = mkprj()
                qa(1)
                if pre_ln1 is not None:
                    bn1(2)
                    bn1(3)
                qa(2)
                prj(0)
                qa(3)
                prj(1)
            else:
                qa(1)
                if pre_ln1 is not None:
                    bn1(2)
                    bn1(3)
                qa(2)
                qa(3)
                # wbig2 holds vT until the last v_build; reload with projT now
                prj = mkprj()
                prj(0)
                prj(1)

            xn2 = [act_p.tile([128, B_CORE, NTOK], BF16, tag=f"act{c}", name=f"xn2_{c}") for c in range(NCH)]
            ls2, bn2 = make_ln(xn2, t0, tl)
            ls2(0)
            prj(2)
            bn2(0)
            bn2(1)
            prj(3)
            ls2(1)
            mlp_nv = make_mlp(L, xn2, t0, tl)
            mlp_nv(0)
            bn2(2)
            bn2(3)
            mlp_nv(1)
            if next_gpsa is None:
                mlp_nv(2)
                mlp_nv(3)
                return None
            nxt = make_ln1(next_gpsa)
            nxt[1](0)          # ls1(0) of next layer
            mlp_nv(2)
            nxt[2](0)          # bn1 nv0
            nxt[2](1)          # bn1 nv1
            mlp_nv(3)          # ls1(1) happens in the next layer, after qa(0)
            return nxt

        layers = [(L, True) for L in gl] + [(L, False) for L in ml]
        pre = None
        for i, (L, gpsa) in enumerate(layers):
            nxt_gpsa = layers[i + 1][1] if i + 1 < len(layers) else None
            pre = emit_layer(L, gpsa, pre, nxt_gpsa)

        # ---- final LN on cls + head -------------------------------------
        hw_sb = w_p.tile([128, NCH, 1024], BF16, tag="wbig")
        nc.sync.dma_start(out=hw_sb, in_=headT.rearrange("(c p) n -> p c n", p=128))
        hb_sb = load_sm(head_b, 8, "fc1b")

        s_ps = pslot()
        q_ps = pslot()
        s_row = s_ps[0:1, 0:B_CORE]
        q_row = q_ps[0:1, 0:B_CORE]
        xbs = []
        for c in range(NCH):
            xb = tmp_p.tile([128, B_CORE], BF16, tag="fxb", name=f"fxb{c}", bufs=6)
            xq = tmp_p.tile([128, B_CORE], BF16, tag="fxq")
            sl = res[c][:, :, 0]
            nc.scalar.activation(out=xb, in_=sl, func=AF.Identity)
            nc.vector.tensor_tensor(out=xq, in0=xb, in1=xb, op=ALU.mult)
            nc.tensor.matmul(s_row, ones_col, xb, start=(c == 0), stop=(c == NCH - 1))
            nc.tensor.matmul(q_row, ones_col, xq, start=(c == 0), stop=(c == NCH - 1))
            xbs.append(xb)
        v1 = row_p.tile([1, B_CORE], F32, tag="fv1")
        nc.vector.tensor_scalar_mul(v1, s_row, 1.0 / C)
        meanb = row_p.tile([1, B_CORE], BF16, tag="fmeanb")
        nc.scalar.activation(out=meanb, in_=v1, func=AF.Identity)
        nc.vector.tensor_tensor(out=v1, in0=v1, in1=v1, op=ALU.mult)
        nc.vector.scalar_tensor_tensor(out=v1, in0=q_row, scalar=1.0 / C,
                                       in1=v1, op0=ALU.mult, op1=ALU.subtract)
        nc.scalar.activation(out=v1, in_=v1, func=AF.Sqrt, bias=eps_sb[0:1, :], scale=1.0)
        recip(v1, v1)
        rstdb = row_p.tile([1, B_CORE], BF16, tag="frstdb")
        nc.vector.tensor_copy(out=rstdb, in_=v1)
        mR = pslot()
        rR = pslot()
        nc.tensor.matmul(mR[:, 0:B_CORE], ones_row, meanb, start=True, stop=True)
        nc.tensor.matmul(rR[:, 0:B_CORE], ones_row, rstdb, start=True, stop=True)
        mRb = mrb_p.tile([128, B_CORE], BF16, tag="fmRb")
        rRb = mrb_p.tile([128, B_CORE], BF16, tag="frRb")
        nc.scalar.activation(out=mRb, in_=mR[:, 0:B_CORE], func=AF.Identity)
        nc.scalar.activation(out=rRb, in_=rR[:, 0:B_CORE], func=AF.Identity)
        xnf = []
        for c in range(NCH):
            tsub = tmp_p.tile([128, B_CORE], BF16, tag="flt")
            nc.vector.tensor_sub(tsub, xbs[c], mRb)
            xc = tmp_p.tile([128, B_CORE], BF16, tag="fxn", name=f"fxn{c}", bufs=6)
            nc.vector.tensor_tensor(out=xc, in0=tsub, in1=rRb, op=ALU.mult)
            xnf.append(xc)
        for mc in range(8):
            ps = pslot()
            psv = ps[:, 0:B_CORE]
            for kc in range(NCH):
                nc.tensor.matmul(psv, hw_sb[:, kc, mc * 128:(mc + 1) * 128], xnf[kc],
                                 start=(kc == 0), stop=(kc == NCH - 1))
            ot = tmp_p.tile([128, B_CORE], F32, tag="fout")
            nc.scalar.activation(out=ot, in_=psv, func=AF.Identity,
                                 bias=hb_sb[:, mc:mc + 1], scale=1.0)
            nc.sync.dma_start(out=OUT[mc * 128:(mc + 1) * 128, :], in_=ot)

    nc.compile()
    return nc


# ---------------------------------------------------------------------------
# host side
# ---------------------------------------------------------------------------

def _rel_nkm():
    ind = np.arange(GRID)[None, :] - np.arange(GRID)[:, None]
    indx = np.tile(ind, (GRID, GRID)).astype(np.float32)
    indy = np.repeat(np.repeat(ind, GRID, axis=0), GRID, axis=1).astype(np.float32)
    indd = indx ** 2 + indy ** 2
    rel = np.stack([indx, indy, indd], axis=0)           # [3, n, m]
    return np.ascontiguousarray(rel.transpose(1, 0, 2))  # [n, 3, m]


def _pcol(v, parts=128):
    v = np.asarray(v, np.float32).reshape(-1, parts)
    return np.ascontiguousarray(v.T)


def _wT(w):
    return np.ascontiguousarray(np.asarray(w, np.float32).T.astype(BF))


def _prep_weights(i):
    d = {}
    d["pwT"] = _wT(np.asarray(i["patch_w"], np.float32).reshape(C, C))
    # pos_embed with patch_b folded in
    posT = np.asarray(i["pos_embed"], np.float32)[0].T.copy()   # [C, N]
    posT += np.asarray(i["patch_b"], np.float32)[:, None]
    d["posT"] = np.ascontiguousarray(posT)
    d["cls"] = np.asarray(i["cls_token"], np.float32).reshape(C)
    ns = np.asarray(i["norm_s"], np.float32)
    nb = np.asarray(i["norm_b"], np.float32)
    hw = np.asarray(i["head_w"], np.float32)               # [1000, C]
    hT = np.zeros((C, 1024), np.float32)
    hT[:, :1000] = (hw * ns[None, :]).T
    d["headT"] = np.ascontiguousarray(hT.astype(BF))
    hb = np.zeros(1024, np.float32)
    hb[:1000] = np.asarray(i["head_b"], np.float32) + hw @ nb
    d["head_b"] = _pcol(hb)
    for l in range(L_G):
        s1 = np.asarray(i["g_norm1_s"][l], np.float32)
        b1 = np.asarray(i["g_norm1_b"][l], np.float32)
        s2 = np.asarray(i["g_norm2_s"][l], np.float32)
        b2 = np.asarray(i["g_norm2_b"][l], np.float32)
        qk = np.asarray(i["g_qk_w"][l], np.float32)        # [2C, C]
        vw = np.asarray(i["g_v_w"][l], np.float32)         # [C, C]
        pw = np.asarray(i["g_proj_w"][l], np.float32)      # [C, C]
        fc1 = np.asarray(i["g_fc1_w"][l], np.float32)      # [4C, C]
        d[f"g{l}_qkT"] = _wT(qk * s1[None, :])
        d[f"g{l}_qkb"] = _pcol(qk @ b1)
        d[f"g{l}_vT"] = _wT(vw * s1[None, :])
        d[f"g{l}_projT"] = _wT(pw)
        d[f"g{l}_projb"] = _pcol(np.asarray(i["g_proj_b"][l], np.float32) + pw @ (vw @ b1))
        d[f"g{l}_fc1T"] = _wT(fc1 * s2[None, :])
        d[f"g{l}_fc1b"] = _pcol(np.asarray(i["g_fc1_b"][l], np.float32) + fc1 @ b2)
        d[f"g{l}_fc2T"] = _wT(i["g_fc2_w"][l])
        d[f"g{l}_fc2b"] = _pcol(i["g_fc2_b"][l])
        # pos-softmax (weight-only): scores[h,n,m] = sum_k rel[n,k,m]*posw[h,k]
        relnkm = _rel_nkm()
        posw = np.asarray(i["g_pos_w"][l], np.float32)
        sc = np.einsum('nkm,hk->hnm', relnkm, posw)
        sc -= sc.max(axis=-1, keepdims=True)
        e = np.exp(sc)
        pos = e / e.sum(axis=-1, keepdims=True)            # [H, N, M]
        ftT = np.ascontiguousarray(pos.transpose(2, 0, 1).reshape(NPATCH, H * NPATCH)
                                   .astype(BF))            # [M, H*N]
        d[f"g{l}_ft"] = ftT
        sig = 1.0 / (1.0 + np.exp(-np.asarray(i["g_gate"][l], np.float32)))
        d[f"g{l}_omdiv"] = np.ascontiguousarray(
            np.tile(np.repeat(1.0 / (1.0 - sig), 64).reshape(1, H * 64), (128, 1)).astype(BF))
        d[f"g{l}_g64"] = np.ascontiguousarray(np.tile(sig.reshape(1, H), (64, 1)))
    for l in range(L_M):
        s1 = np.asarray(i["m_norm1_s"][l], np.float32)
        b1 = np.asarray(i["m_norm1_b"][l], np.float32)
        s2 = np.asarray(i["m_norm2_s"][l], np.float32)
        b2 = np.asarray(i["m_norm2_b"][l], np.float32)
        qkv = np.asarray(i["m_qkv_w"][l], np.float32)      # [3C, C]
        pw = np.asarray(i["m_proj_w"][l], np.float32)
        fc1 = np.asarray(i["m_fc1_w"][l], np.float32)
        d[f"m{l}_qkvT"] = _wT(qkv * s1[None, :])
        d[f"m{l}_qkb"] = _pcol(qkv[:2 * C] @ b1)
        d[f"m{l}_projT"] = _wT(pw)
        d[f"m{l}_projb"] = _pcol(np.asarray(i["m_proj_b"][l], np.float32)
                                 + pw @ (qkv[2 * C:] @ b1))
        d[f"m{l}_fc1T"] = _wT(fc1 * s2[None, :])
        d[f"m{l}_fc1b"] = _pcol(np.asarray(i["m_fc1_b"][l], np.float32) + fc1 @ b2)
        d[f"m{l}_fc2T"] = _wT(i["m_fc2_w"][l])
        d[f"m{l}_fc2b"] = _pcol(i["m_fc2_b"][l])
    return d


_last_results = None


def build_in_maps(inputs):
    wmap = _prep_weights(inputs)
    x = np.asarray(inputs["x"], np.float32)
    in_maps = []
    for core in range(NCORES):
        xs = x[core * B_CORE:(core + 1) * B_CORE]
        xi = xs.reshape(B_CORE, 3, GRID, PS, GRID, PS).transpose(1, 3, 5, 0, 2, 4)
        xi = np.ascontiguousarray(xi.reshape(C, B_CORE * NPATCH).astype(BF))
        m = dict(wmap)
        m["xim"] = xi
        in_maps.append(m)
    return in_maps


def get_program():
    if "nc" not in _CACHE:
        _CACHE["nc"] = _build_program()
    return _CACHE["nc"]


def kernel(**inputs):
    global _last_results
    _install_ntff_hook()
    from concourse import bass_utils

    nc = get_program()
    in_maps = build_in_maps(inputs)
    res = bass_utils.run_bass_kernel_spmd(nc, in_maps, core_ids=list(range(NCORES)))
    _last_results = res
    outs = [r["out"][:1000, :].T for r in res.results]
    return np.ascontiguousarray(np.concatenate(outs, axis=0).astype(np.float32))



# revision 40
# speedup vs baseline: 1.2039x; 1.0507x over previous
"""ConViT (12-layer, H=12, D=64, B=64) forward pass on 8 TRN2 NeuronCores.

Strategy: data-parallel over batch (8 images per core). Host does layout prep
only (im2col of the non-overlapping patch conv, weight transposes to [ci, co],
bf16 casts, LayerNorm affine folding into consumer weights); all FLOPs run
on-device. Activations are feature-major [C -> 6x128 partitions, tokens free];
matmuls are bf16 with f32 PSUM; residual stream in f32.

Perf notes vs v1:
 - LN affine (s, b) folded into qk/v/fc1/head weights on host; V-path bias
   eliminated entirely (softmax rows sum to 1) and absorbed into proj bias.
 - proj/fc2 bias folded into the PSUM->residual scalar_tensor_tensor epilogue.
 - reciprocal_approx_fast instead of iterative reciprocal.
 - single persistent PSUM ring pool (no per-phase pool barriers).
 - attention: merged exp over both key halves; softmax denominator broadcast
   into rows 64:128 of the same PSUM slot (col-tiled matmul); one-head
   lookahead pipeline.
"""
import os
import sys
import types
import contextlib
import ctypes
from contextlib import ExitStack

import numpy as np
import ml_dtypes

import concourse.bass as bass
import concourse.mybir as mybir
import concourse.tile as tile
from concourse import bacc
from concourse.masks import make_identity

F32 = mybir.dt.float32
BF16 = mybir.dt.bfloat16
AF = mybir.ActivationFunctionType
ALU = mybir.AluOpType
BF = ml_dtypes.bfloat16

H = 12
D = 64
C = 768
NCH = 6             # C / 128
PS = 16             # patch size
GRID = 14
NPATCH = 196        # tokens per image in GPSA phase
NTOK = 197          # tokens per image in MHSA phase (cls + patches)
L_G = 3
L_M = 9
SCALE = D ** -0.5
EPS = 1e-5
B_CORE = 8
NCORES = 8
MLPC = 24           # 3072 / 128

_CACHE = {}
FAST_RECIP = False


def _install_ntff_hook():
    """Best-effort: enable NTFF profiling under axon (used when BASS_TRACE=1)."""
    if "antenv.axon_hooks" in sys.modules:
        return
    so_path = "/opt/axon/libaxon_pjrt.so"
    if not os.path.exists(so_path):
        return
    try:
        lib = ctypes.CDLL(so_path)
        if not hasattr(lib, "axon_start_nrt_profile"):
            return
        lib.axon_start_nrt_profile.argtypes = [ctypes.POINTER(ctypes.c_int64), ctypes.c_size_t]
        lib.axon_start_nrt_profile.restype = ctypes.c_int64
        lib.axon_stop_nrt_profile.argtypes = [ctypes.c_char_p]
        lib.axon_stop_nrt_profile.restype = ctypes.c_int64

        @contextlib.contextmanager
        def _hook(output_dir, device_ids):
            import jax
            jax.devices()
            if device_ids:
                ids = (ctypes.c_int64 * len(device_ids))(*device_ids)
                rc = lib.axon_start_nrt_profile(ids, len(device_ids))
            else:
                rc = lib.axon_start_nrt_profile(None, 0)
            if rc != 0:
                raise RuntimeError(f"axon_start_nrt_profile rc={rc}")
            try:
                yield
            finally:
                n = lib.axon_stop_nrt_profile(str(output_dir).encode())
                if n < 0:
                    raise RuntimeError(f"axon_stop_nrt_profile rc={n}")

        mod = types.ModuleType("antenv.axon_hooks")
        mod._hook = _hook
        mod.get_axon_ntff_profile_hook = lambda: mod._hook
        mod.set_axon_ntff_profile_hook = lambda h: setattr(mod, "_hook", h)
        sys.modules["antenv.axon_hooks"] = mod
        import antenv
        antenv.axon_hooks = mod
    except Exception:
        pass


def _build_program():
    nc = bacc.Bacc("TRN2", target_bir_lowering=False, debug=False)

    def din(name, shape, dt):
        return nc.dram_tensor(name, shape, dt, kind="ExternalInput").ap()

    xim = din("xim", [C, B_CORE * NPATCH], BF16)
    pwT = din("pwT", [C, C], BF16)
    posT = din("posT", [C, NPATCH], F32)           # pos_embed + patch_b folded
    cls = din("cls", [C], F32)
    headT = din("headT", [C, 1024], BF16)          # * norm_s folded
    head_b = din("head_b", [128, 8], F32)          # + headT @ norm_b folded

    gl, ml = [], []
    for i in range(L_G):
        gl.append({
            "qkT": din(f"g{i}_qkT", [C, 2 * C], BF16),      # * n1s folded
            "qkb": din(f"g{i}_qkb", [128, 2 * NCH], F32),   # qk_w @ n1b
            "vT": din(f"g{i}_vT", [C, C], BF16),            # * n1s folded
            "projT": din(f"g{i}_projT", [C, C], BF16),
            "projb": din(f"g{i}_projb", [128, NCH], F32),   # + proj_w @ (v_w @ n1b)
            "fc1T": din(f"g{i}_fc1T", [C, 4 * C], BF16),    # * n2s folded
            "fc1b": din(f"g{i}_fc1b", [128, MLPC], F32),    # + fc1_w @ n2b
            "fc2T": din(f"g{i}_fc2T", [4 * C, C], BF16),
            "fc2b": din(f"g{i}_fc2b", [128, NCH], F32),
            "ft": din(f"g{i}_ft", [NPATCH, H * NPATCH], BF16),  # pos-softmax^T
            "omdiv": din(f"g{i}_omdiv", [128, H * 64], BF16),   # 1/(1-sigmoid(gate))
            "g64": din(f"g{i}_g64", [64, H], F32),              # sigmoid(gate) bcast
        })
    for i in range(L_M):
        ml.append({
            "qkvT": din(f"m{i}_qkvT", [C, 3 * C], BF16),    # * n1s folded
            "qkb": din(f"m{i}_qkb", [128, 2 * NCH], F32),   # qk rows of qkv_w @ n1b
            "projT": din(f"m{i}_projT", [C, C], BF16),
            "projb": din(f"m{i}_projb", [128, NCH], F32),   # + proj_w @ (v_w @ n1b)
            "fc1T": din(f"m{i}_fc1T", [C, 4 * C], BF16),
            "fc1b": din(f"m{i}_fc1b", [128, MLPC], F32),
            "fc2T": din(f"m{i}_fc2T", [4 * C, C], BF16),
            "fc2b": din(f"m{i}_fc2b", [128, NCH], F32),
        })

    OUT = nc.dram_tensor("out", [1024, B_CORE], F32, kind="ExternalOutput").ap()

    MIDTAGS = [f"qt{j}" for j in range(NCH)] + [f"kt{j}" for j in range(NCH)]

    with ExitStack() as ctx:
        tc = ctx.enter_context(tile.TileContext(nc))

        consts = ctx.enter_context(tc.tile_pool(name="consts", bufs=1))
        res_p = ctx.enter_context(tc.tile_pool(name="res", bufs=1))
        act_p = ctx.enter_context(tc.tile_pool(name="act", bufs=2))   # xn / OT / xn2
        qk_p = ctx.enter_context(tc.tile_pool(name="qk", bufs=2))     # Q^T/K^T + mids + xb
        v_p = ctx.enter_context(tc.tile_pool(name="vp", bufs=1))      # token-major V
        w_p = ctx.enter_context(tc.tile_pool(name="wp", bufs=1))      # streamed weights
        wsm_p = ctx.enter_context(tc.tile_pool(name="wsm", bufs=2))   # per-layer params
        row_p = ctx.enter_context(tc.tile_pool(name="rows", bufs=2))  # [1, *] stat rows
        rowa_p = ctx.enter_context(tc.tile_pool(name="rowsa", bufs=2))  # attn recip rows
        mrb_p = ctx.enter_context(tc.tile_pool(name="mrb", bufs=2))   # bcast mean/rstd bf16
        tmp_p = ctx.enter_context(tc.tile_pool(name="tmp", bufs=2))   # scratch tiles
        tmp1_p = ctx.enter_context(tc.tile_pool(name="tmp1", bufs=2))  # big f32 scratch
        e_p = ctx.enter_context(tc.tile_pool(name="ep", bufs=4))      # exp tiles
        ft_p = ctx.enter_context(tc.tile_pool(name="ft", bufs=1))     # GPSA pos F^T

        # single persistent PSUM pool: "ps" ring of 6 one-bank slots + "mmv"
        psum = ctx.enter_context(tc.tile_pool(name="psum", bufs=6, space="PSUM"))

        def pslot():
            # exactly one 2KB PSUM bank -> slots stay bank-aligned
            return psum.tile([128, 512], F32, tag="ps", name="ps")

        def pv(t):
            # [128, 2, NTOK] view of a bank slot
            return t[:, 0:2 * NTOK].rearrange("p (b t) -> p b t", t=NTOK)

        def recip(out, in_):
            if FAST_RECIP:
                nc.vector.reciprocal_approx_fast(out=out, in_=in_)
            else:
                nc.vector.reciprocal(out=out, in_=in_)

        def _scalar_act_raw(out, in_, func, bias):
            # InstActivation without bass's Reciprocal/Rsqrt accuracy guard.
            eng = nc.scalar
            ins = [eng.lower_ap(in_)]
            for arg in (bias, 1.0, 0.0):  # bias, scale, alpha
                if isinstance(arg, bass.AP):
                    ins.append(eng.lower_ap(arg))
                else:
                    ins.append(mybir.ImmediateValue(dtype=F32, value=arg))
            return eng.add_instruction(mybir.InstActivation(
                name=eng.bass.get_next_instruction_name(),
                func=func, ins=ins, outs=[eng.lower_ap(out)]))

        def scalar_recip(out, in_):
            return _scalar_act_raw(out, in_, AF.Reciprocal, 0.0)

        def scalar_rsqrt(out, in_, bias_ap):
            return _scalar_act_raw(out, in_, AF.Rsqrt, bias_ap)

        ones_col = consts.tile([128, 1], BF16)
        nc.vector.memset(ones_col, 1.0)
        ones_row = consts.tile([1, 128], BF16)
        nc.vector.memset(ones_row, 1.0)
        ones64 = consts.tile([128, 64], BF16)
        nc.vector.memset(ones64, 1.0)
        eps_sb = consts.tile([128, 1], F32)
        nc.vector.memset(eps_sb, EPS)

        res = [res_p.tile([128, B_CORE, NTOK], F32, tag=f"res{c}", name=f"res{c}") for c in range(NCH)]

        # persistent token-major V tiles
        vtiles = {}
        for bi in range(2):
            for hi in range(2):
                vsb = v_p.tile([128, H, 64], BF16, tag=f"v{bi}{hi}", name=f"v{bi}{hi}")
                vtiles[(bi, hi)] = vsb

        # cls token into res[:, :, 0]
        for c in range(NCH):
            src = cls[c * 128:(c + 1) * 128]
            ap = bass.AP(tensor=src.tensor, offset=src.offset,
                         ap=[list(src.ap[0]), [0, B_CORE], [0, 1]])
            nc.sync.dma_start(out=res[c][:, :, 0:1], in_=ap)

        def load_wT(dram, ncol, tag):
            t = w_p.tile([128, NCH, ncol], BF16, tag=tag)
            nc.sync.dma_start(out=t, in_=dram.rearrange("(c p) n -> p c n", p=128))
            return t

        def load_sm(dram, ncol, tag, dt=F32):
            t = wsm_p.tile([128, ncol], dt, tag=tag)
            nc.sync.dma_start(out=t, in_=dram)
            return t

        # ---- patch embed -------------------------------------------------
        pw_sb = load_wT(pwT, C, "wbig")
        xim_sb = w_p.tile([128, MLPC, C], BF16, tag="wbig2")  # reuse fc2-size slot
        ximv = xim_sb.rearrange("p a b -> p (a b)")[:, 0:NCH * B_CORE * NPATCH] \
            .rearrange("p (c n) -> p c n", c=NCH)
        ximd = xim.rearrange("(c p) n -> p c n", p=128)
        for nv in range(4):
            sl = slice(2 * nv * NPATCH, (2 * nv + 2) * NPATCH)
            nc.sync.dma_start(out=ximv[:, :, sl], in_=ximd[:, :, sl])

        for nv in range(4):
            b0 = 2 * nv
            for mc in range(NCH):
                ps = pslot()
                psf = ps[:, 0:2 * NPATCH]
                for kc in range(NCH):
                    nc.tensor.matmul(
                        psf, pw_sb[:, kc, mc * 128:(mc + 1) * 128],
                        ximv[:, kc, b0 * NPATCH:(b0 + 2) * NPATCH],
                        start=(kc == 0), stop=(kc == NCH - 1))
                nc.scalar.activation(
                    out=res[mc][:, b0:b0 + 2, 1:NTOK],
                    in_=psf.rearrange("p (b t) -> p b t", b=2),
                    func=AF.Identity, scale=1.0)
        for c in range(NCH):
            src = posT[c * 128:(c + 1) * 128]
            ap = bass.AP(tensor=src.tensor, offset=src.offset,
                         ap=[list(src.ap[0]), [0, B_CORE], list(src.ap[1])])
            nc.gpsimd.dma_start(out=res[c][:, :, 1:NTOK], in_=ap, accum_op=ALU.add)

        # ---- helpers -----------------------------------------------------
        def make_ln(xn, t0, tl):
            """res -> xn bf16 normalized (no affine; folded into consumers).
            Returns (ls, bn): ls(pair) emits stats + row chains for image
            pairs 2p,2p+1; bn(pair) emits broadcast + normalize. Callers
            weave these between dense phases so the serial row chain hides
            behind matmul work."""
            ntl = 2 * tl
            chains = {}

            def stats(nv):
                b0 = 2 * nv
                s_ps = pslot()
                q_ps = pslot()
                s_row = s_ps[0:1, 0:ntl]
                q_row = q_ps[0:1, 0:ntl]
                for c in range(NCH):
                    xb = qk_p.tile([128, 2, tl], BF16, tag=MIDTAGS[c], name=f"xb{c}")
                    xq = tmp_p.tile([128, 2, tl], BF16, tag="xq")
                    sl = res[c][:, b0:b0 + 2, t0:t0 + tl]
                    nc.scalar.activation(out=xb, in_=sl, func=AF.Identity)
                    nc.vector.tensor_tensor(out=xq, in0=xb, in1=xb, op=ALU.mult)
                    nc.tensor.matmul(s_row, ones_col, xb.rearrange("p b t -> p (b t)"),
                                     start=(c == 0), stop=(c == NCH - 1))
                    nc.tensor.matmul(q_row, ones_col, xq.rearrange("p b t -> p (b t)"),
                                     start=(c == 0), stop=(c == NCH - 1))
                return s_row, q_row

            def rowchain(s_row, q_row):
                v1 = row_p.tile([1, ntl], F32, tag="v1")
                nc.vector.tensor_scalar_mul(v1, s_row, 1.0 / C)
                meanb = row_p.tile([1, ntl], BF16, tag="meanb")
                nc.scalar.activation(out=meanb, in_=v1, func=AF.Identity)
                nc.vector.tensor_tensor(out=v1, in0=v1, in1=v1, op=ALU.mult)
                nc.vector.scalar_tensor_tensor(out=v1, in0=q_row, scalar=1.0 / C,
                                               in1=v1, op0=ALU.mult, op1=ALU.subtract)
                nc.scalar.activation(out=v1, in_=v1, func=AF.Sqrt,
                                     bias=eps_sb[0:1, :], scale=1.0)
                nc.vector.reciprocal_approx_fast(out=v1, in_=v1)
                rstdb = row_p.tile([1, ntl], BF16, tag="rstdb")
                nc.scalar.activation(out=rstdb, in_=v1, func=AF.Identity)
                return meanb, rstdb

            def bcast_norm(nv, meanb, stdb):
                b0 = 2 * nv
                mR = pslot()
                rR = pslot()
                mRf = mR[:, 0:ntl]
                rRf = rR[:, 0:ntl]
                nc.tensor.matmul(mRf, ones_row, meanb, start=True, stop=True)
                nc.tensor.matmul(rRf, ones_row, stdb, start=True, stop=True)
                mRb = mrb_p.tile([128, 2, tl], BF16, tag="mRb")
                rRb = mrb_p.tile([128, 2, tl], BF16, tag="rRb")
                nc.scalar.activation(out=mRb.rearrange("p b t -> p (b t)"), in_=mRf,
                                     func=AF.Identity)
                nc.scalar.activation(out=rRb.rearrange("p b t -> p (b t)"), in_=rRf,
                                     func=AF.Identity)
                for c in range(NCH):
                    tsub = tmp_p.tile([128, 2, tl], BF16, tag="lnt")
                    nc.vector.tensor_sub(tsub, res[c][:, b0:b0 + 2, t0:t0 + tl], mRb)
                    nc.vector.tensor_tensor(out=xn[c][:, b0:b0 + 2, t0:t0 + tl],
                                            in0=tsub, in1=rRb, op=ALU.mult)

            def ls(pair):
                st = [stats(2 * pair + i) for i in range(2)]
                chains[pair] = [rowchain(s, q) for s, q in st]

            def bn(nv):
                bcast_norm(nv, *chains[nv // 2][nv % 2])

            return ls, bn

        def make_proj(wT_sb, biast, t0, tl, rhs_of):
            """proj(nv): res += (rhs @ W^T) + bias."""
            def proj(nv):
                b0 = 2 * nv
                for mc in range(NCH):
                    ps = pslot()
                    psf = ps[:, 0:2 * tl]
                    for kc in range(NCH):
                        nc.tensor.matmul(
                            psf, wT_sb[:, kc, mc * 128:(mc + 1) * 128], rhs_of(kc, b0),
                            start=(kc == 0), stop=(kc == NCH - 1))
                    sl = res[mc][:, b0:b0 + 2, t0:t0 + tl]
                    nc.vector.scalar_tensor_tensor(
                        out=sl, in0=psf.rearrange("p (b t) -> p b t", b=2),
                        scalar=biast[:, mc:mc + 1], in1=sl,
                        op0=ALU.add, op1=ALU.add)
            return proj

        def make_mlp(L, xn, t0, tl):
            fc1_sb = load_wT(L["fc1T"], 4 * C, "wbig")
            fc1b_sb = load_sm(L["fc1b"], MLPC, "fc1b")
            fc2_sb = w_p.tile([128, MLPC, C], BF16, tag="wbig2")
            nc.sync.dma_start(out=fc2_sb, in_=L["fc2T"].rearrange("(c p) n -> p c n", p=128))
            fc2b_sb = load_sm(L["fc2b"], NCH, "fc2b")

            def mlp_nv(nv):
                b0 = 2 * nv
                mids = []
                for mc in range(MLPC):
                    ps = pslot()
                    psf = ps[:, 0:2 * tl]
                    for kc in range(NCH):
                        nc.tensor.matmul(
                            psf, fc1_sb[:, kc, mc * 128:(mc + 1) * 128],
                            xn[kc][:, b0:b0 + 2, t0:t0 + tl],
                            start=(kc == 0), stop=(kc == NCH - 1))
                    mt = qk_p.tile([128, 2 * NTOK], BF16, tag=MIDTAGS[mc % 12])
                    nc.scalar.activation(out=mt[:, 0:2 * tl], in_=psf, func=AF.Gelu,
                                         bias=fc1b_sb[:, mc:mc + 1], scale=1.0)
                    mids.append(mt)
                for mc in range(NCH):
                    ps = pslot()
                    psf = ps[:, 0:2 * tl]
                    for kc in range(MLPC):
                        nc.tensor.matmul(
                            psf, fc2_sb[:, kc, mc * 128:(mc + 1) * 128],
                            mids[kc][:, 0:2 * tl],
                            start=(kc == 0), stop=(kc == MLPC - 1))
                    sl = res[mc][:, b0:b0 + 2, t0:t0 + tl]
                    nc.vector.scalar_tensor_tensor(
                        out=sl, in0=psf.rearrange("p (b t) -> p b t", b=2),
                        scalar=fc2b_sb[:, mc:mc + 1], in1=sl,
                        op0=ALU.add, op1=ALU.add)
            return mlp_nv

        def make_qa(L, xn, OT, t0, tl, gpsa, pos_ctx):
            """qa(nv): QKV projections -> V build -> attention -> OT."""
            kl = tl - 128
            nkeys = [(0, 128), (128, kl)]
            w_qk = pos_ctx["w_qk"]
            w_v = pos_ctx["w_v"]
            qkb_sb = pos_ctx["qkb"]

            def qa(nv):
                b0 = 2 * nv
                qt = [qk_p.tile([128, 2, NTOK], BF16, tag=f"qt{c}", name=f"qtt{c}") for c in range(NCH)]
                kt = [qk_p.tile([128, 2, NTOK], BF16, tag=f"kt{c}", name=f"ktt{c}") for c in range(NCH)]
                for mc in range(2 * NCH):
                    ps = pslot()
                    psf = ps[:, 0:2 * tl]
                    for kc in range(NCH):
                        nc.tensor.matmul(
                            psf, w_qk[:, kc, mc * 128:(mc + 1) * 128],
                            xn[kc][:, b0:b0 + 2, t0:t0 + tl],
                            start=(kc == 0), stop=(kc == NCH - 1))
                    dst = qt[mc] if mc < NCH else kt[mc - NCH]
                    nc.scalar.activation(
                        out=dst[:, :, 0:tl],
                        in_=psf.rearrange("p (b t) -> p b t", b=2), func=AF.Identity,
                        bias=qkb_sb[:, mc:mc + 1], scale=1.0)
                vt = vtiles
                for bi in range(2):
                    b = b0 + bi
                    for hi, (h0, hl) in enumerate(nkeys):
                        vsb = vt[(bi, hi)]
                        ps = psum.tile([128, C], F32, tag="mmv", bufs=1, name="psv",
                                       padded_shape=[128, 1024])
                        for kc in range(NCH):
                            for c0, cl in ((0, 512), (512, 256)):
                                nc.tensor.matmul(
                                    ps[:hl, c0:c0 + cl],
                                    xn[kc][:, b, t0 + h0:t0 + h0 + hl],
                                    w_v(kc)[:, c0:c0 + cl],
                                    start=(kc == 0), stop=(kc == NCH - 1))
                        nc.scalar.activation(
                            out=vsb[:hl, :, 0:64],
                            in_=ps[:hl].rearrange("p (h d) -> p h d", h=H),
                            func=AF.Identity)

                def avden(h, es):
                    """AV matmuls + ones/omdiv denominator broadcast."""
                    oe = pv(pslot())
                    for bi in range(2):
                        nc.tensor.matmul(oe[0:64, bi, 0:tl], vt[(bi, 0)][:, h, 0:64],
                                         es[bi][:, 0, 0:tl], start=(bi == 0), stop=False)
                        nc.tensor.matmul(oe[0:64, bi, 0:tl], vt[(bi, 1)][:kl, h, 0:64],
                                         es[bi][:kl, 1, 0:tl], start=False, stop=(bi == 1))
                    db = pv(pslot())
                    dlhs = pos_ctx["omdiv"] if gpsa else None
                    for bi in range(2):
                        nc.tensor.matmul(db[0:64, bi, 0:tl],
                                         dlhs[:, h, :] if gpsa else ones64,
                                         es[bi][:, 0, 0:tl], start=(bi == 0), stop=False)
                        nc.tensor.matmul(db[0:64, bi, 0:tl],
                                         dlhs[0:kl, h, :] if gpsa else ones64[0:kl],
                                         es[bi][:kl, 1, 0:tl], start=False, stop=(bi == 1))
                    return oe, db

                def normalize(h, oedb, fp):
                    oe, db = oedb
                    ch, off = h // 2, (h % 2) * 64
                    r_sb = rowa_p.tile([64, 2, NTOK], F32, tag="db", bufs=2)
                    nc.vector.reciprocal_approx_fast(out=r_sb[:, :, 0:tl],
                                                     in_=db[0:64, :, 0:tl])
                    for bi in range(2):
                        b = b0 + bi
                        if gpsa:
                            tf = tmp1_p.tile([64, NTOK], BF16, tag="tf")
                            nc.vector.tensor_tensor(out=tf[:, 0:tl],
                                                    in0=oe[0:64, bi, 0:tl],
                                                    in1=r_sb[:, bi, 0:tl], op=ALU.mult)
                            nc.vector.scalar_tensor_tensor(
                                out=OT[ch][off:off + 64, b, t0:t0 + tl],
                                in0=fp[0:64, bi, 0:tl], scalar=pos_ctx["g64"][:, h:h + 1],
                                in1=tf[:, 0:tl], op0=ALU.mult, op1=ALU.add)
                        else:
                            nc.vector.tensor_tensor(
                                out=OT[ch][off:off + 64, b, t0:t0 + tl],
                                in0=oe[0:64, bi, 0:tl], in1=r_sb[:, bi, 0:tl],
                                op=ALU.mult)

                if gpsa:
                    def stageA(h):
                        ch, off = h // 2, (h % 2) * 64
                        es = []
                        for bi in range(2):
                            s = pv(pslot())
                            nc.tensor.matmul(s[:, 0, 0:tl],
                                             kt[ch][off:off + 64, bi, 0:128],
                                             qt[ch][off:off + 64, bi, 0:tl],
                                             start=True, stop=True)
                            nc.tensor.matmul(s[:kl, 1, 0:tl],
                                             kt[ch][off:off + 64, bi, 128:tl],
                                             qt[ch][off:off + 64, bi, 0:tl],
                                             start=True, stop=True)
                            e = e_p.tile([128, 2, NTOK], BF16, tag="e", name="e",
                                         bufs=8)
                            # rows kl:128 of the second half are stale garbage;
                            # never read downstream.
                            nc.scalar.activation(out=e[:, :, 0:tl], in_=s[:, :, 0:tl],
                                                 func=AF.Exp, scale=SCALE)
                            es.append(e)
                        return es

                    def stageB(h, es):
                        FT = pos_ctx["FT"]
                        fp = pv(pslot())
                        for bi in range(2):
                            nc.tensor.matmul(fp[0:64, bi, 0:tl], vt[(bi, 0)][:, h, 0:64],
                                             FT[0][:, h, :], start=True, stop=False)
                            nc.tensor.matmul(fp[0:64, bi, 0:tl], vt[(bi, 1)][:kl, h, 0:64],
                                             FT[1][:kl, h, :], start=False, stop=True)
                        oe = avden(h, es)
                        normalize(h, oe, fp)

                    prev = None
                    for h in range(H):
                        es = stageA(h)
                        if prev is not None:
                            stageB(*prev)
                        prev = (h, es)
                    stageB(*prev)
                else:
                    # MHSA: head pairs (2j, 2j+1) live at partition offsets 0/64
                    # of chunk j -> row-tiled score matmuls run concurrently.
                    def stageA(j):
                        ss = {}
                        for idx in range(2):
                            for bi in range(2):
                                ss[(idx, bi)] = pv(pslot())
                        for bi in range(2):
                            for ci, (c0, cl_) in enumerate(((0, 128), (128, kl))):
                                for idx, off in ((0, 0), (1, 64)):
                                    s = ss[(idx, bi)]
                                    nc.tensor.matmul(
                                        s[0:cl_, ci, 0:tl],
                                        kt[j][off:off + 64, bi, c0:c0 + cl_],
                                        qt[j][off:off + 64, bi, 0:tl],
                                        start=True, stop=True)
                        es = {}
                        for idx in range(2):
                            for bi in range(2):
                                e = e_p.tile([128, 2, NTOK], BF16, tag="e", name="e",
                                             bufs=8)
                                nc.scalar.activation(out=e[:, :, 0:tl],
                                                     in_=ss[(idx, bi)][:, :, 0:tl],
                                                     func=AF.Exp, scale=SCALE)
                                es[(idx, bi)] = e
                        return es

                    def stageB(j, es):
                        for idx in range(2):
                            h = 2 * j + idx
                            epair = [es[(idx, 0)], es[(idx, 1)]]
                            oe = avden(h, epair)
                            normalize(h, oe, None)

                    prev = None
                    for j in range(H // 2):
                        es = stageA(j)
                        if prev is not None:
                            stageB(*prev)
                        prev = (j, es)
                    stageB(*prev)

            return qa

        def make_ln1(gpsa):
            t0, tl = (1, NPATCH) if gpsa else (0, NTOK)
            xn = [act_p.tile([128, B_CORE, NTOK], BF16, tag=f"act{c}", name=f"xn{c}")
                  for c in range(NCH)]
            ls1, bn1 = make_ln(xn, t0, tl)
            return xn, ls1, bn1

        def emit_layer(L, gpsa, pre_ln1, next_gpsa):
            """Emit one transformer layer, weaving LN stages between dense
            phases so their serial row chains hide behind PE matmul work.
            pre_ln1: (xn, ls1, bn1) with ls1(0) and bn1(0) already emitted
            by the previous layer's tail; bn1(1..3)/ls1(1) still pending.
            next_gpsa: None at the last layer, else next layer's gpsa flag;
            returns next layer's pre_ln1."""
            t0, tl = (1, NPATCH) if gpsa else (0, NTOK)
            projb_sb = load_sm(L["projb"], NCH, "projb")
            qkb_sb = load_sm(L["qkb"], 2 * NCH, "qkb")

            pos_ctx = {"qkb": qkb_sb}
            if gpsa:
                pos_ctx["w_qk"] = load_wT(L["qkT"], 2 * C, "wbig")
                v_sb = load_wT(L["vT"], C, "wbig2")
                pos_ctx["w_v"] = lambda kc: v_sb[:, kc, :]
                # host-precomputed pos-softmax^T [key m, head, query n]
                FT = [ft_p.tile([128, H, NPATCH], BF16, tag=f"ft{i}", name=f"ft{i}") for i in range(2)]
                pos_ctx["FT"] = FT
                ftd = L["ft"].rearrange("m (h n) -> m h n", h=H)
                nc.sync.dma_start(out=FT[0][:128], in_=ftd[0:128])
                nc.sync.dma_start(out=FT[1][:68], in_=ftd[128:196])
                omdiv = wsm_p.tile([128, H, 64], BF16, tag="omdiv")
                nc.sync.dma_start(out=omdiv, in_=L["omdiv"].rearrange("p (h d) -> p h d", h=H))
                pos_ctx["omdiv"] = omdiv
                g64 = wsm_p.tile([64, H], F32, tag="g64")
                nc.sync.dma_start(out=g64, in_=L["g64"])
                pos_ctx["g64"] = g64
            else:
                qkv_sb = load_wT(L["qkvT"], 3 * C, "wbig")
                pos_ctx["w_qk"] = qkv_sb
                pos_ctx["w_v"] = lambda kc: qkv_sb[:, kc, 2 * C:3 * C]

            if pre_ln1 is None:
                xn, ls1, bn1 = make_ln1(gpsa)
                ls1(0)
                bn1(0)
                bn1(1)
                ls1(1)
                bn1(2)
                bn1(3)
            else:
                xn, ls1, bn1 = pre_ln1

            OT = [act_p.tile([128, B_CORE, NTOK], BF16, tag=f"act{c}", name=f"ot{c}") for c in range(NCH)]
            qa = make_qa(L, xn, OT, t0, tl, gpsa, pos_ctx)

            def mkprj():
                proj_sb = load_wT(L["projT"], C, "wbig2")
                return make_proj(proj_sb, projb_sb, t0, tl,
                                 lambda kc, b0: OT[kc][:, b0:b0 + 2, t0:t0 + tl])

            qa(0)
            if pre_ln1 is not None:
                bn1(2)
                bn1(3)
            if not gpsa:
                # proj weights go to wbig2 (free since last layer's fc2)
                prj = mkprj()
                qa(1)
                qa(2)
                prj(0)
                qa(3)
                prj(1)
            else:
                qa(1)
                qa(2)
                qa(3)
                # wbig2 holds vT until the last v_build; reload with projT now
                prj = mkprj()
                prj(0)
                prj(1)

            xn2 = [act_p.tile([128, B_CORE, NTOK], BF16, tag=f"act{c}", name=f"xn2_{c}") for c in range(NCH)]
            ls2, bn2 = make_ln(xn2, t0, tl)
            ls2(0)
            prj(2)
            bn2(0)
            bn2(1)
            prj(3)
            ls2(1)
            mlp_nv = make_mlp(L, xn2, t0, tl)
            mlp_nv(0)
            bn2(2)
            bn2(3)
            mlp_nv(1)
            if next_gpsa is None:
                mlp_nv(2)
                mlp_nv(3)
                return None
            nxt = make_ln1(next_gpsa)
            nxt[1](0)          # ls1(0) of next layer
            mlp_nv(2)
            nxt[2](0)          # bn1 nv0
            nxt[2](1)          # bn1 nv1
            mlp_nv(3)
            nxt[1](1)          # ls1(1)
            return nxt

        layers = [(L, True) for L in gl] + [(L, False) for L in ml]
        pre = None
        for i, (L, gpsa) in enumerate(layers):
            nxt_gpsa = layers[i + 1][1] if i + 1 < len(layers) else None
            pre = emit_layer(L, gpsa, pre, nxt_gpsa)

        # ---- final LN on cls + head -------------------------------------
        hw_sb = w_p.tile([128, NCH, 1024], BF16, tag="wbig")
        nc.sync.dma_start(out=hw_sb, in_=headT.rearrange("(c p) n -> p c n", p=128))
        hb_sb = load_sm(head_b, 8, "fc1b")

        s_ps = pslot()
        q_ps = pslot()
        s_row = s_ps[0:1, 0:B_CORE]
        q_row = q_ps[0:1, 0:B_CORE]
        xbs = []
        for c in range(NCH):
            xb = tmp_p.tile([128, B_CORE], BF16, tag="fxb", name=f"fxb{c}", bufs=6)
            xq = tmp_p.tile([128, B_CORE], BF16, tag="fxq")
            sl = res[c][:, :, 0]
            nc.scalar.activation(out=xb, in_=sl, func=AF.Identity)
            nc.vector.tensor_tensor(out=xq, in0=xb, in1=xb, op=ALU.mult)
            nc.tensor.matmul(s_row, ones_col, xb, start=(c == 0), stop=(c == NCH - 1))
            nc.tensor.matmul(q_row, ones_col, xq, start=(c == 0), stop=(c == NCH - 1))
            xbs.append(xb)
        v1 = row_p.tile([1, B_CORE], F32, tag="fv1")
        nc.vector.tensor_scalar_mul(v1, s_row, 1.0 / C)
        meanb = row_p.tile([1, B_CORE], BF16, tag="fmeanb")
        nc.scalar.activation(out=meanb, in_=v1, func=AF.Identity)
        nc.vector.tensor_tensor(out=v1, in0=v1, in1=v1, op=ALU.mult)
        nc.vector.scalar_tensor_tensor(out=v1, in0=q_row, scalar=1.0 / C,
                                       in1=v1, op0=ALU.mult, op1=ALU.subtract)
        nc.scalar.activation(out=v1, in_=v1, func=AF.Sqrt, bias=eps_sb[0:1, :], scale=1.0)
        recip(v1, v1)
        rstdb = row_p.tile([1, B_CORE], BF16, tag="frstdb")
        nc.vector.tensor_copy(out=rstdb, in_=v1)
        mR = pslot()
        rR = pslot()
        nc.tensor.matmul(mR[:, 0:B_CORE], ones_row, meanb, start=True, stop=True)
        nc.tensor.matmul(rR[:, 0:B_CORE], ones_row, rstdb, start=True, stop=True)
        mRb = mrb_p.tile([128, B_CORE], BF16, tag="fmRb")
        rRb = mrb_p.tile([128, B_CORE], BF16, tag="frRb")
        nc.scalar.activation(out=mRb, in_=mR[:, 0:B_CORE], func=AF.Identity)
        nc.scalar.activation(out=rRb, in_=rR[:, 0:B_CORE], func=AF.Identity)
        xnf = []
        for c in range(NCH):
            tsub = tmp_p.tile([128, B_CORE], BF16, tag="flt")
            nc.vector.tensor_sub(tsub, xbs[c], mRb)
            xc = tmp_p.tile([128, B_CORE], BF16, tag="fxn", name=f"fxn{c}", bufs=6)
            nc.vector.tensor_tensor(out=xc, in0=tsub, in1=rRb, op=ALU.mult)
            xnf.append(xc)
        for mc in range(8):
            ps = pslot()
            psv = ps[:, 0:B_CORE]
            for kc in range(NCH):
                nc.tensor.matmul(psv, hw_sb[:, kc, mc * 128:(mc + 1) * 128], xnf[kc],
                                 start=(kc == 0), stop=(kc == NCH - 1))
            ot = tmp_p.tile([128, B_CORE], F32, tag="fout")
            nc.scalar.activation(out=ot, in_=psv, func=AF.Identity,
                                 bias=hb_sb[:, mc:mc + 1], scale=1.0)
            nc.sync.dma_start(out=OUT[mc * 128:(mc + 1) * 128, :], in_=ot)

    nc.compile()
    return nc


# ---------------------------------------------------------------------------
# host side
# ---------------------------------------------------------------------------

def _rel_nkm():
    ind = np.arange(GRID)[None, :] - np.arange(GRID)[:, None]
    indx = np.tile(ind, (GRID, GRID)).astype(np.float32)
    indy = np.repeat(np.repeat(ind, GRID, axis=0), GRID, axis=1).astype(np.float32)
    indd = indx ** 2 + indy ** 2
    rel = np.stack([indx, indy, indd], axis=0)           # [3, n, m]
    return np.ascontiguousarray(rel.transpose(1, 0, 2))  # [n, 3, m]


def _pcol(v, parts=128):
    v = np.asarray(v, np.float32).reshape(-1, parts)
    return np.ascontiguousarray(v.T)


def _wT(w):
    return np.ascontiguousarray(np.asarray(w, np.float32).T.astype(BF))


def _prep_weights(i):
    d = {}
    d["pwT"] = _wT(np.asarray(i["patch_w"], np.float32).reshape(C, C))
    # pos_embed with patch_b folded in
    posT = np.asarray(i["pos_embed"], np.float32)[0].T.copy()   # [C, N]
    posT += np.asarray(i["patch_b"], np.float32)[:, None]
    d["posT"] = np.ascontiguousarray(posT)
    d["cls"] = np.asarray(i["cls_token"], np.float32).reshape(C)
    ns = np.asarray(i["norm_s"], np.float32)
    nb = np.asarray(i["norm_b"], np.float32)
    hw = np.asarray(i["head_w"], np.float32)               # [1000, C]
    hT = np.zeros((C, 1024), np.float32)
    hT[:, :1000] = (hw * ns[None, :]).T
    d["headT"] = np.ascontiguousarray(hT.astype(BF))
    hb = np.zeros(1024, np.float32)
    hb[:1000] = np.asarray(i["head_b"], np.float32) + hw @ nb
    d["head_b"] = _pcol(hb)
    for l in range(L_G):
        s1 = np.asarray(i["g_norm1_s"][l], np.float32)
        b1 = np.asarray(i["g_norm1_b"][l], np.float32)
        s2 = np.asarray(i["g_norm2_s"][l], np.float32)
        b2 = np.asarray(i["g_norm2_b"][l], np.float32)
        qk = np.asarray(i["g_qk_w"][l], np.float32)        # [2C, C]
        vw = np.asarray(i["g_v_w"][l], np.float32)         # [C, C]
        pw = np.asarray(i["g_proj_w"][l], np.float32)      # [C, C]
        fc1 = np.asarray(i["g_fc1_w"][l], np.float32)      # [4C, C]
        d[f"g{l}_qkT"] = _wT(qk * s1[None, :])
        d[f"g{l}_qkb"] = _pcol(qk @ b1)
        d[f"g{l}_vT"] = _wT(vw * s1[None, :])
        d[f"g{l}_projT"] = _wT(pw)
        d[f"g{l}_projb"] = _pcol(np.asarray(i["g_proj_b"][l], np.float32) + pw @ (vw @ b1))
        d[f"g{l}_fc1T"] = _wT(fc1 * s2[None, :])
        d[f"g{l}_fc1b"] = _pcol(np.asarray(i["g_fc1_b"][l], np.float32) + fc1 @ b2)
        d[f"g{l}_fc2T"] = _wT(i["g_fc2_w"][l])
        d[f"g{l}_fc2b"] = _pcol(i["g_fc2_b"][l])
        # pos-softmax (weight-only): scores[h,n,m] = sum_k rel[n,k,m]*posw[h,k]
        relnkm = _rel_nkm()
        posw = np.asarray(i["g_pos_w"][l], np.float32)
        sc = np.einsum('nkm,hk->hnm', relnkm, posw)
        sc -= sc.max(axis=-1, keepdims=True)
        e = np.exp(sc)
        pos = e / e.sum(axis=-1, keepdims=True)            # [H, N, M]
        ftT = np.ascontiguousarray(pos.transpose(2, 0, 1).reshape(NPATCH, H * NPATCH)
                                   .astype(BF))            # [M, H*N]
        d[f"g{l}_ft"] = ftT
        sig = 1.0 / (1.0 + np.exp(-np.asarray(i["g_gate"][l], np.float32)))
        d[f"g{l}_omdiv"] = np.ascontiguousarray(
            np.tile(np.repeat(1.0 / (1.0 - sig), 64).reshape(1, H * 64), (128, 1)).astype(BF))
        d[f"g{l}_g64"] = np.ascontiguousarray(np.tile(sig.reshape(1, H), (64, 1)))
    for l in range(L_M):
        s1 = np.asarray(i["m_norm1_s"][l], np.float32)
        b1 = np.asarray(i["m_norm1_b"][l], np.float32)
        s2 = np.asarray(i["m_norm2_s"][l], np.float32)
        b2 = np.asarray(i["m_norm2_b"][l], np.float32)
        qkv = np.asarray(i["m_qkv_w"][l], np.float32)      # [3C, C]
        pw = np.asarray(i["m_proj_w"][l], np.float32)
        fc1 = np.asarray(i["m_fc1_w"][l], np.float32)
        d[f"m{l}_qkvT"] = _wT(qkv * s1[None, :])
        d[f"m{l}_qkb"] = _pcol(qkv[:2 * C] @ b1)
        d[f"m{l}_projT"] = _wT(pw)
        d[f"m{l}_projb"] = _pcol(np.asarray(i["m_proj_b"][l], np.float32)
                                 + pw @ (qkv[2 * C:] @ b1))
        d[f"m{l}_fc1T"] = _wT(fc1 * s2[None, :])
        d[f"m{l}_fc1b"] = _pcol(np.asarray(i["m_fc1_b"][l], np.float32) + fc1 @ b2)
        d[f"m{l}_fc2T"] = _wT(i["m_fc2_w"][l])
        d[f"m{l}_fc2b"] = _pcol(i["m_fc2_b"][l])
    return d


_last_results = None


def build_in_maps(inputs):
    wmap = _prep_weights(inputs)
    x = np.asarray(inputs["x"], np.float32)
    in_maps = []
    for core in range(NCORES):
        xs = x[core * B_CORE:(core + 1) * B_CORE]
        xi = xs.reshape(B_CORE, 3, GRID, PS, GRID, PS).transpose(1, 3, 5, 0, 2, 4)
        xi = np.ascontiguousarray(xi.reshape(C, B_CORE * NPATCH).astype(BF))
        m = dict(wmap)
        m["xim"] = xi
        in_maps.append(m)
    return in_maps


def get_program():
    if "nc" not in _CACHE:
        _CACHE["nc"] = _build_program()
    return _CACHE["nc"]


def kernel(**inputs):
    global _last_results
    _install_ntff_hook()
    from concourse import bass_utils

    nc = get_program()
    in_maps = build_in_maps(inputs)
    res = bass_utils.run_bass_kernel_spmd(nc, in_maps, core_ids=list(range(NCORES)))
    _last_results = res
    outs = [r["out"][:1000, :].T for r in res.results]
    return np.ascontiguousarray(np.concatenate(outs, axis=0).astype(np.float32))

